# revision 3
# baseline (speedup 1.0000x reference)
"""Causal self-attention (GPT-style, B=2, T=4096, C=768, 12 heads) on 8 TRN2
NeuronCores.

Sharding: core c handles batch b = c//4 and heads [3g, 3g+1, 3g+2] with
g = c%4 (data parallel on B x tensor parallel on heads).  Each core computes
its heads' attention output projected through its slice of w_proj; the host
sums the 4 partial [T, C] outputs per batch and adds b_proj.

Device-side formulation (all matmuls bf16, fp32 accumulate):
  - host passes x[b].T so QKV projections contract C on partitions:
      qT/kT  = W.T @ x.T        -> [head_dim(=partitions), T]
      V'     = x @ [Wv|0] + ones-col -> [T(=partitions), 3*65]  (col 64 of
               each 65-block is constant 1 -> PV also yields softmax denoms)
  - scores computed transposed, S^T[k, q] = (K chunk) @ (Q chunk)^T via
    lhsT=kT, rhs=qT; two heads per 512-cycle slot via PE row-tiling (K=64).
  - softmax without max-subtraction (scores are O(5), exp is safe in fp32):
      P^T = exp(0.125 * S^T)  on ScalarE, PSUM->SBUF bf16, one instruction
      per [128, 1024] (two heads / two k-chunks share one activate).
  - causal masking: strictly-masked k-chunks never computed; boundary tiles
    multiplied by precomputed 0/1 bf16 masks on VectorE after exp.
  - PV: oT'[65, q] += V'[k,65].T @ P^T[k,q] accumulated over k-chunks; row 64
    is the softmax denominator.  Normalize: recip on DVE, broadcast across 64
    partitions via a rank-1 PE matmul, multiply -> yT[h] [64, T] bf16.
  - output projection: out[t, c] = sum_h yT[h][:, t].T @ w_proj[h-rows, c].
"""

import numpy as np

N_CORES = 8
B = 2
T = 4096
C = 768
NH = 12
HD = 64
HPC = 3            # heads per core
TCH = 512          # t / q chunk
KCH = 128          # k chunk
CPART = 128

_cache = {}


def _ensure_axon_hooks_module():
    """Make `from antenv.axon_hooks import ...` importable even on images
    whose antenv package lacks the module (profiling then degrades to a
    no-op instead of crashing run_bass_kernel_spmd(trace=True))."""
    import sys
    import types
    try:
        import antenv.axon_hooks  # noqa: F401
        return
    except Exception:
        pass
    m = types.ModuleType("antenv.axon_hooks")
    m._hook = None

    def _set(h):
        m._hook = h

    def _get():
        return m._hook

    m.set_axon_ntff_profile_hook = _set
    m.get_axon_ntff_profile_hook = _get
    sys.modules["antenv.axon_hooks"] = m


def build_program(t=T):
    """Build the single-core SPMD bass program (same program on all cores,
    per-core data). Returns the un-finalized Bacc."""
    import concourse.mybir as mybir
    import concourse.tile as tile
    from concourse import bacc
    from concourse.bass import ds, ts

    f32 = mybir.dt.float32
    bf16 = mybir.dt.bfloat16
    AF = mybir.ActivationFunctionType

    nt = t // TCH          # number of t/q chunks
    nkc_tot = t // KCH     # number of k chunks
    cc_n = C // CPART      # 6 contraction chunks

    nc = bacc.Bacc("TRN2", target_bir_lowering=False)

    xT = nc.dram_tensor("xT", [C, t], f32, kind="ExternalInput")
    wq01_d = nc.dram_tensor("wq01", [C, 128], f32, kind="ExternalInput")
    wk01_d = nc.dram_tensor("wk01", [C, 128], f32, kind="ExternalInput")
    wq2d_d = nc.dram_tensor("wq2d", [C, 128], f32, kind="ExternalInput")
    wk2d_d = nc.dram_tensor("wk2d", [C, 128], f32, kind="ExternalInput")
    wv_d = nc.dram_tensor("wv", [C, 195], f32, kind="ExternalInput")
    bv1_d = nc.dram_tensor("bv1", [1, 195], bf16, kind="ExternalInput")
    bq01_d = nc.dram_tensor("bq01", [128, 1], f32, kind="ExternalInput")
    bk01_d = nc.dram_tensor("bk01", [128, 1], f32, kind="ExternalInput")
    bq2d_d = nc.dram_tensor("bq2d", [128, 1], f32, kind="ExternalInput")
    bk2d_d = nc.dram_tensor("bk2d", [128, 1], f32, kind="ExternalInput")
    wp_d = nc.dram_tensor("wp", [HPC * HD, C], f32, kind="ExternalInput")
    m01_d = nc.dram_tensor("m01", [4, 128, 1024], bf16, kind="ExternalInput")
    m2_d = nc.dram_tensor("m2", [2, 128, 1024], bf16, kind="ExternalInput")
    ones128_d = nc.dram_tensor("ones128", [1, 128], bf16, kind="ExternalInput")
    ones64_d = nc.dram_tensor("ones64", [1, 64], f32, kind="ExternalInput")
    out_d = nc.dram_tensor("out", [t, C], f32, kind="ExternalOutput")

    with tile.TileContext(nc) as tc_:
        with (
            tc_.tile_pool(name="consts", bufs=1) as consts,
            tc_.tile_pool(name="big", bufs=1) as big,
            tc_.tile_pool(name="xin", bufs=2) as xin,
            tc_.tile_pool(name="ptp", bufs=4) as ptp,
            tc_.tile_pool(name="wkp", bufs=3) as wkp,
        ):
            # ---- init: weights / biases / masks to SBUF (bf16) ----
            def load_w(nm, dram, shape_f32, rearr, **kw):
                stage = xin.tile(list(shape_f32), f32, tag="wstage",
                                 name="wstage")
                nc.sync.dma_start(stage[:], dram[:, :].rearrange(rearr, **kw))
                wsb = consts.tile(list(shape_f32), bf16, name=nm, tag=nm)
                nc.vector.tensor_copy(wsb[:], stage[:])
                return wsb

            wq01_sb = load_w("wq01_sb", wq01_d, [128, cc_n, 128],
                             "(po pi) m -> pi po m", pi=128)
            wk01_sb = load_w("wk01_sb", wk01_d, [128, cc_n, 128],
                             "(po pi) m -> pi po m", pi=128)
            wq2d_sb = load_w("wq2d_sb", wq2d_d, [128, cc_n, 128],
                             "(po pi) m -> pi po m", pi=128)
            wk2d_sb = load_w("wk2d_sb", wk2d_d, [128, cc_n, 128],
                             "(po pi) m -> pi po m", pi=128)
            wv_sb = load_w("wv_sb", wv_d, [128, cc_n, 195],
                           "(po pi) m -> pi po m", pi=128)
            wp_sb = load_w("wp_sb", wp_d, [64, HPC, C], "(h j) c -> j h c",
                           j=64)

            bq01_sb = consts.tile([128, 1], f32)
            bk01_sb = consts.tile([128, 1], f32)
            bq2d_sb = consts.tile([128, 1], f32)
            bk2d_sb = consts.tile([128, 1], f32)
            nc.sync.dma_start(bq01_sb[:], bq01_d[:, :])
            nc.sync.dma_start(bk01_sb[:], bk01_d[:, :])
            nc.sync.dma_start(bq2d_sb[:], bq2d_d[:, :])
            nc.sync.dma_start(bk2d_sb[:], bk2d_d[:, :])
            bv1_sb = consts.tile([1, 195], bf16)
            nc.sync.dma_start(bv1_sb[:], bv1_d[:, :])
            ones128_sb = consts.tile([1, 128], bf16)
            nc.sync.dma_start(ones128_sb[:], ones128_d[:, :])
            ones64_sb = consts.tile([1, 64], f32)
            nc.sync.dma_start(ones64_sb[:], ones64_d[:, :])
            m01_sb = consts.tile([128, 4, 1024], bf16)
            nc.sync.dma_start(m01_sb[:], m01_d[:, :, :].rearrange(
                "m p j -> p m j"))
            m2_sb = consts.tile([128, 2, 1024], bf16)
            nc.sync.dma_start(m2_sb[:], m2_d[:, :, :].rearrange(
                "m p j -> p m j"))

            # ---- persistent activations ----
            Q01 = big.tile([128, t], bf16)   # rows 0-63 qT_h0, 64-127 qT_h1
            K01 = big.tile([128, t], bf16)
            Q2 = big.tile([128, t], bf16)    # qT_h2 duplicated on both halves
            K2 = big.tile([128, t], bf16)
            Vp = big.tile([128, nkc_tot, 195], bf16)  # V' tiles per k-chunk
            yT0 = big.tile([64, t], bf16)
            yT1 = big.tile([64, t], bf16)
            yT2 = big.tile([64, t], bf16)
            yT = [yT0, yT1, yT2]

            xT_r = xT[:, :].rearrange("(po pi) t -> pi po t", pi=128)

            with (
                tc_.tile_pool(name="sps", bufs=2, space="PSUM") as sps,
                tc_.tile_pool(name="ops", bufs=3, space="PSUM") as ops,
            ):
                # ---- phase 1: QKV projections ----
                for tci in range(nt):
                    xt32 = xin.tile([128, cc_n, TCH], f32, tag="xt32",
                                    name="xt32")
                    nc.sync.dma_start(xt32[:], xT_r[:, :, ts(tci, TCH)])
                    xtb = xin.tile([128, cc_n, TCH], bf16, tag="xtb",
                                   name="xtb")
                    nc.vector.tensor_copy(xtb[:], xt32[:])
                    for wsb, bsb, dst in (
                        (wq01_sb, bq01_sb, Q01),
                        (wk01_sb, bk01_sb, K01),
                        (wq2d_sb, bq2d_sb, Q2),
                        (wk2d_sb, bk2d_sb, K2),
                    ):
                        qkps = sps.tile([128, TCH], f32, tag="S", name="qkps")
                        for cc in range(cc_n):
                            nc.tensor.matmul(
                                qkps[:], wsb[:, cc, :], xtb[:, cc, :],
                                start=(cc == 0), stop=(cc == cc_n - 1))
                        nc.vector.tensor_scalar_add(
                            dst[:, ts(tci, TCH)], qkps[:], bsb[:])
                    for st in range(TCH // 128):
                        tt = tci * (TCH // 128) + st
                        vps = ops.tile([128, 195], f32, tag="oT", name="vps")
                        for cc in range(cc_n):
                            nc.tensor.matmul(
                                vps[:], xtb[:, cc, ts(st, 128)],
                                wv_sb[:, cc, :],
                                start=(cc == 0), stop=False)
                        nc.tensor.matmul(vps[:], ones128_sb[:], bv1_sb[:],
                                         start=False, stop=True)
                        nc.vector.tensor_copy(Vp[:, tt, :], vps[:])

                # ---- phase 2: attention ----
                def normalize(oT, h, qc):
                    recip = wkp.tile([1, TCH], f32, tag="recip", name="recip")
                    nc.vector.reciprocal(recip[:], oT[64:65, :])
                    rbps = sps.tile([64, TCH], f32, tag="S", name="rbps")
                    nc.tensor.matmul(rbps[:], ones64_sb[:], recip[:],
                                     start=True, stop=True)
                    rb = wkp.tile([64, TCH], f32, tag="rb", name="rb")
                    nc.vector.tensor_copy(rb[:], rbps[:])
                    nc.vector.tensor_mul(yT[h][:, ts(qc, TCH)], oT[0:64, :],
                                         rb[:])

                for qc in range(nt):
                    nkc = (qc + 1) * (TCH // KCH)
                    # pass 1: heads 0,1 row-tiled
                    oT0 = ops.tile([65, TCH], f32, tag="oT", name="oT0")
                    oT1 = ops.tile([65, TCH], f32, tag="oT", name="oT1")
                    s_pend = {}

                    def emit_s01(kc):
                        S = sps.tile([128, 1024], f32, tag="S", name="S01")
                        nc.tensor.matmul(
                            S[:, 0:TCH],
                            K01[0:64, ts(kc, KCH)], Q01[0:64, ts(qc, TCH)],
                            start=True, stop=True, tile_position=(0, 0))
                        nc.tensor.matmul(
                            S[:, TCH:1024],
                            K01[64:128, ts(kc, KCH)], Q01[64:128, ts(qc, TCH)],
                            start=True, stop=True, tile_position=(64, 0))
                        s_pend[kc] = S

                    emit_s01(0)
                    for kc in range(nkc):
                        S = s_pend.pop(kc)
                        PT = ptp.tile([128, 1024], bf16, tag="PT", name="PT")
                        nc.scalar.activation(PT[:], S[:], AF.Exp, scale=0.125)
                        if kc + 1 < nkc:
                            emit_s01(kc + 1)
                        m = kc - (qc * (TCH // KCH))
                        if m >= 0:
                            nc.vector.tensor_mul(PT[:], PT[:], m01_sb[:, m, :])
                        nc.tensor.matmul(oT0[:], Vp[:, kc, 0:65], PT[:, 0:TCH],
                                         start=(kc == 0), stop=(kc == nkc - 1))
                        nc.tensor.matmul(oT1[:], Vp[:, kc, 65:130],
                                         PT[:, TCH:1024],
                                         start=(kc == 0), stop=(kc == nkc - 1))
                    normalize(oT0, 0, qc)
                    normalize(oT1, 1, qc)

                    # pass 2: head 2, k-chunk pairs row-tiled
                    oT2 = ops.tile([65, TCH], f32, tag="oT", name="oT2")
                    npair = nkc // 2
                    s2_pend = {}

                    def emit_s2(kp):
                        kc0, kc1 = 2 * kp, 2 * kp + 1
                        S2 = sps.tile([128, 1024], f32, tag="S", name="S2")
                        nc.tensor.matmul(
                            S2[:, 0:TCH],
                            K2[0:64, ts(kc0, KCH)], Q2[0:64, ts(qc, TCH)],
                            start=True, stop=True, tile_position=(0, 0))
                        nc.tensor.matmul(
                            S2[:, TCH:1024],
                            K2[64:128, ts(kc1, KCH)], Q2[64:128, ts(qc, TCH)],
                            start=True, stop=True, tile_position=(64, 0))
                        s2_pend[kp] = S2

                    emit_s2(0)
                    for kp in range(npair):
                        kc0, kc1 = 2 * kp, 2 * kp + 1
                        S2 = s2_pend.pop(kp)
                        PT2 = ptp.tile([128, 1024], bf16, tag="PT", name="PT2")
                        nc.scalar.activation(PT2[:], S2[:], AF.Exp, scale=0.125)
                        if kp + 1 < npair:
                            emit_s2(kp + 1)
                        m = kc0 - (qc * (TCH // KCH))
                        if m >= 0:
                            nc.vector.tensor_mul(PT2[:], PT2[:],
                                                 m2_sb[:, m // 2, :])
                        nc.tensor.matmul(oT2[:], Vp[:, kc0, 130:195],
                                         PT2[:, 0:TCH],
                                         start=(kp == 0), stop=False)
                        nc.tensor.matmul(oT2[:], Vp[:, kc1, 130:195],
                                         PT2[:, TCH:1024],
                                         start=False, stop=(kp == npair - 1))
                    normalize(oT2, 2, qc)

            # ---- phase 3: output projection ----
            with tc_.tile_pool(name="prps", bufs=2, space="PSUM") as prps:
                for tt in range(t // 128):
                    po = prps.tile([128, C], f32, tag="po", name="po")
                    for h in range(HPC):
                        for cs, cw in ((0, 512), (512, 256)):
                            nc.tensor.matmul(
                                po[:, ds(cs, cw)],
                                yT[h][:, ts(tt, 128)],
                                wp_sb[:, h, ds(cs, cw)],
                                start=(h == 0), stop=(h == HPC - 1))
                    pout = xin.tile([128, C], f32, tag="pout", name="pout")
                    nc.vector.tensor_copy(pout[:], po[:])
                    nc.sync.dma_start(out_d[ts(tt, 128), :], pout[:])

    return nc


def make_masks():
    import ml_dtypes
    p = np.arange(128)[:, None]
    j = np.arange(TCH)[None, :]

    def tri(shift):
        return (j - p - shift >= 0).astype(np.float32)

    m01 = np.stack([np.concatenate([tri(128 * m)] * 2, axis=1)
                    for m in range(4)])
    m2 = np.stack([np.concatenate([tri(128 * m), tri(128 * (m + 1))], axis=1)
                   for m in (0, 2)])
    return (m01.astype(ml_dtypes.bfloat16), m2.astype(ml_dtypes.bfloat16))


def core_inputs(c, x, w_attn, b_attn, w_proj, xT_by_batch, m01, m2):
    import ml_dtypes
    f32 = np.float32
    b = c // 4
    heads = [(c % 4) * HPC + i for i in range(HPC)]
    h0, h1, h2 = heads

    def Wq(h):
        return w_attn[:, h * HD:(h + 1) * HD]

    def Wk(h):
        return w_attn[:, C + h * HD:C + (h + 1) * HD]

    def Wv(h):
        return w_attn[:, 2 * C + h * HD:2 * C + (h + 1) * HD]

    def bq(h):
        return b_attn[h * HD:(h + 1) * HD]

    def bk(h):
        return b_attn[C + h * HD:C + (h + 1) * HD]

    def bv(h):
        return b_attn[2 * C + h * HD:2 * C + (h + 1) * HD]

    wv195 = np.zeros((C, 195), f32)
    bv1 = np.zeros((1, 195), f32)
    for i, h in enumerate(heads):
        wv195[:, i * 65:i * 65 + 64] = Wv(h)
        bv1[0, i * 65:i * 65 + 64] = bv(h)
        bv1[0, i * 65 + 64] = 1.0
    return {
        "xT": xT_by_batch[b],
        "wq01": np.ascontiguousarray(np.concatenate([Wq(h0), Wq(h1)], 1)),
        "wk01": np.ascontiguousarray(np.concatenate([Wk(h0), Wk(h1)], 1)),
        "wq2d": np.ascontiguousarray(np.concatenate([Wq(h2), Wq(h2)], 1)),
        "wk2d": np.ascontiguousarray(np.concatenate([Wk(h2), Wk(h2)], 1)),
        "wv": wv195,
        "bv1": bv1.astype(ml_dtypes.bfloat16),
        "bq01": np.ascontiguousarray(
            np.concatenate([bq(h0), bq(h1)])[:, None]),
        "bk01": np.ascontiguousarray(
            np.concatenate([bk(h0), bk(h1)])[:, None]),
        "bq2d": np.ascontiguousarray(
            np.concatenate([bq(h2), bq(h2)])[:, None]),
        "bk2d": np.ascontiguousarray(
            np.concatenate([bk(h2), bk(h2)])[:, None]),
        "wp": np.ascontiguousarray(
            np.concatenate([w_proj[h * HD:(h + 1) * HD, :] for h in heads],
                           0)),
        "m01": m01,
        "m2": m2,
        "ones128": np.ones((1, 128), ml_dtypes.bfloat16),
        "ones64": np.ones((1, 64), f32),
    }


TRACE = False
LAST_EXEC_NS = None
LAST_RESULTS = None


def kernel(x, w_attn, b_attn, w_proj, b_proj):
    global LAST_EXEC_NS, LAST_RESULTS
    _ensure_axon_hooks_module()
    from concourse.bass_utils import run_bass_kernel_spmd

    x = np.asarray(x, np.float32)
    w_attn = np.asarray(w_attn, np.float32)
    b_attn = np.asarray(b_attn, np.float32)
    w_proj = np.asarray(w_proj, np.float32)
    b_proj = np.asarray(b_proj, np.float32)

    if "nc" not in _cache:
        nc = build_program()
        nc.finalize()
        _cache["nc"] = nc
    nc = _cache["nc"]

    xT_by_batch = [np.ascontiguousarray(x[b].T) for b in range(B)]
    m01, m2 = make_masks()
    in_maps = [
        core_inputs(c, x, w_attn, b_attn, w_proj, xT_by_batch, m01, m2)
        for c in range(N_CORES)
    ]
    res = run_bass_kernel_spmd(nc, in_maps, core_ids=list(range(N_CORES)),
                               trace=TRACE)
    LAST_EXEC_NS = res.exec_time_ns
    LAST_RESULTS = res
    out = np.zeros((B, T, C), np.float32)
    for c in range(N_CORES):
        out[c // 4] += np.asarray(res.results[c]["out"], np.float32)
    out += b_proj[None, None, :]
    return out


# revision 6
# speedup vs baseline: 1.5773x; 1.5773x over previous
"""Causal self-attention (GPT-style, B=2, T=4096, C=768, 12 heads) on 8 TRN2
NeuronCores.

Sharding: core c handles batch b = c//4 and heads [3g, 3g+1, 3g+2] with
g = c%4 (data parallel on B x tensor parallel on heads).  Each core computes
its heads' attention output projected through its slice of w_proj; the host
sums the 4 partial [T, C] outputs per batch and adds b_proj.

Device-side formulation (all matmuls bf16, fp32 accumulate):
  - host passes x[b].T so QKV projections contract C on partitions:
      qT/kT  = W.T @ x.T        -> [head_dim(=partitions), T]
      V'     = x @ [Wv|0] + ones-col -> [T(=partitions), 3*65]  (col 64 of
               each 65-block is constant 1 -> PV also yields softmax denoms)
  - scores computed transposed, S^T[k, q] via lhsT=kT, rhs=qT; two heads per
    512-cycle slot via PE row-tiling (K=64 each, concurrent row groups).
  - softmax without max-subtraction (scores are O(5), exp safe in fp32):
      P^T = exp(0.125 * S^T) on ScalarE, PSUM->SBUF bf16, one activate per
      [128, 1024] (both heads / both k-chunks of a slot share it).
  - causal: strictly-masked k-chunks never computed; at boundary tiles the
    score matmul / exp / PV shrink to the valid column subrange and only the
    diagonal [128,128] strip is multiplied by a triangular 0/1 bf16 mask.
  - PV: oT'[65, q] += V'[k,65].T @ P^T[k,q] accumulated over k-chunks; row 64
    is the softmax denominator.  Normalize: approx-reciprocal on DVE,
    partition-broadcast via a stride-0 SBUF->SBUF DMA, multiply -> yT bf16.
  - output projection (packed K=128): out[t,:] = yTa[:,t].T @ wp[h01-rows] +
    yT2[:,t].T @ wp[h2-rows];  h1's normalized slab is DMA-shifted into
    partitions 64..127 of yTa so two heads contract in one matmul.
  - QKV-projection and output-projection work is interleaved into the
    attention loop as PE filler ops so TensorE never idles (keeps the HAM
    clock gate at 2.4 GHz) while ScalarE streams the exps.
"""

import numpy as np

N_CORES = 8
B = 2
T = 4096
C = 768
NH = 12
HD = 64
HPC = 3            # heads per core
TCH = 512          # t / q chunk
KCH = 128          # k chunk
CPART = 128

_cache = {}


def _ensure_axon_hooks_module():
    """Make `from antenv.axon_hooks import ...` importable even on images
    whose antenv package lacks the module (profiling then degrades to a
    no-op instead of crashing run_bass_kernel_spmd(trace=True))."""
    import sys
    import types
    try:
        import antenv.axon_hooks  # noqa: F401
        return
    except Exception:
        pass
    m = types.ModuleType("antenv.axon_hooks")
    m._hook = None

    def _set(h):
        m._hook = h

    def _get():
        return m._hook

    m.set_axon_ntff_profile_hook = _set
    m.get_axon_ntff_profile_hook = _get
    sys.modules["antenv.axon_hooks"] = m


def build_program(t=T):
    """Build the single-core SPMD bass program (same program on all cores,
    per-core data). Returns the un-finalized Bacc."""
    import concourse.mybir as mybir
    import concourse.tile as tile
    from concourse import bacc
    from concourse.bass import ds, ts

    f32 = mybir.dt.float32
    bf16 = mybir.dt.bfloat16
    AF = mybir.ActivationFunctionType

    nt = t // TCH          # number of t/q chunks
    spk = TCH // KCH       # k-chunks per t-chunk (4)
    cc_n = C // CPART      # 6 contraction chunks

    nc = bacc.Bacc("TRN2", target_bir_lowering=False)

    xT = nc.dram_tensor("xT", [C, t], f32, kind="ExternalInput")
    wq01_d = nc.dram_tensor("wq01", [C, 128], f32, kind="ExternalInput")
    wk01_d = nc.dram_tensor("wk01", [C, 128], f32, kind="ExternalInput")
    wq2d_d = nc.dram_tensor("wq2d", [C, 128], f32, kind="ExternalInput")
    wk2d_d = nc.dram_tensor("wk2d", [C, 128], f32, kind="ExternalInput")
    wv_d = nc.dram_tensor("wv", [C, 195], f32, kind="ExternalInput")
    bv1_d = nc.dram_tensor("bv1", [1, 195], bf16, kind="ExternalInput")
    bq01_d = nc.dram_tensor("bq01", [128, 1], f32, kind="ExternalInput")
    bk01_d = nc.dram_tensor("bk01", [128, 1], f32, kind="ExternalInput")
    bq2d_d = nc.dram_tensor("bq2d", [128, 1], f32, kind="ExternalInput")
    bk2d_d = nc.dram_tensor("bk2d", [128, 1], f32, kind="ExternalInput")
    wp_d = nc.dram_tensor("wp", [HPC * HD, C], f32, kind="ExternalInput")
    tri_d = nc.dram_tensor("tri", [128, 128], bf16, kind="ExternalInput")
    ones128_d = nc.dram_tensor("ones128", [1, 128], bf16, kind="ExternalInput")
    out_d = nc.dram_tensor("out", [t, C], f32, kind="ExternalOutput")

    with tile.TileContext(nc) as tc_:
        with (
            tc_.tile_pool(name="consts", bufs=1) as consts,
            tc_.tile_pool(name="big", bufs=1) as big,
            tc_.tile_pool(name="xin", bufs=2) as xin,
            tc_.tile_pool(name="ptp", bufs=4) as ptp,
            tc_.tile_pool(name="wkp", bufs=3) as wkp,
            tc_.tile_pool(name="sps", bufs=2, space="PSUM") as sps,
            tc_.tile_pool(name="ops", bufs=4, space="PSUM") as ops,
        ):
            # ---- init: weights / biases / masks to SBUF (bf16) ----
            def load_w(nm, dram_ap, shape_f32):
                stage = xin.tile(list(shape_f32), f32, tag="wstage",
                                 name="wstage")
                nc.sync.dma_start(stage[:], dram_ap)
                wsb = consts.tile(list(shape_f32), bf16, name=nm, tag=nm)
                nc.vector.tensor_copy(wsb[:], stage[:])
                return wsb

            rr = "(po pi) m -> pi po m"
            wq01_sb = load_w("wq01_sb", wq01_d[:, :].rearrange(rr, pi=128),
                             [128, cc_n, 128])
            wk01_sb = load_w("wk01_sb", wk01_d[:, :].rearrange(rr, pi=128),
                             [128, cc_n, 128])
            wq2d_sb = load_w("wq2d_sb", wq2d_d[:, :].rearrange(rr, pi=128),
                             [128, cc_n, 128])
            wk2d_sb = load_w("wk2d_sb", wk2d_d[:, :].rearrange(rr, pi=128),
                             [128, cc_n, 128])
            wv_sb = load_w("wv_sb", wv_d[:, :].rearrange(rr, pi=128),
                           [128, cc_n, 195])
            wpA_sb = load_w("wpA_sb", wp_d[0:128, :], [128, C])
            wpB_sb = load_w("wpB_sb", wp_d[128:192, :], [64, C])

            bq01_sb = consts.tile([128, 1], f32)
            bk01_sb = consts.tile([128, 1], f32)
            bq2d_sb = consts.tile([128, 1], f32)
            bk2d_sb = consts.tile([128, 1], f32)
            nc.sync.dma_start(bq01_sb[:], bq01_d[:, :])
            nc.sync.dma_start(bk01_sb[:], bk01_d[:, :])
            nc.sync.dma_start(bq2d_sb[:], bq2d_d[:, :])
            nc.sync.dma_start(bk2d_sb[:], bk2d_d[:, :])
            bv1_sb = consts.tile([1, 195], bf16)
            nc.sync.dma_start(bv1_sb[:], bv1_d[:, :])
            ones128_sb = consts.tile([1, 128], bf16)
            nc.sync.dma_start(ones128_sb[:], ones128_d[:, :])
            tri_sb = consts.tile([128, 128], bf16)
            nc.sync.dma_start(tri_sb[:], tri_d[:, :])

            # ---- persistent activations ----
            Q01 = big.tile([128, t], bf16)   # rows 0-63 qT_h0, 64-127 qT_h1
            K01 = big.tile([128, t], bf16)
            Q2 = big.tile([128, t], bf16)    # qT_h2 duplicated on both halves
            K2 = big.tile([128, t], bf16)
            Vp = big.tile([128, t // KCH, 195], bf16)
            yTa = big.tile([128, t], bf16)   # normalized h0 (0:64) | h1
            yT2 = big.tile([64, t], bf16)

            xT_r = xT[:, :].rearrange("(po pi) t -> pi po t", pi=128)

            # ---- QKV projection ops for one t-chunk (list of closures) ----
            def qkv_ops(tci):
                state = {}
                ops_l = []

                def dma_cast():
                    xt32 = xin.tile([128, cc_n, TCH], f32, tag="xt32",
                                    name="xt32")
                    nc.sync.dma_start(xt32[:], xT_r[:, :, ts(tci, TCH)])
                    xtb = xin.tile([128, cc_n, TCH], bf16, tag="xtb",
                                   name="xtb")
                    nc.vector.tensor_copy(xtb[:], xt32[:])
                    state["xtb"] = xtb
                ops_l.append(dma_cast)

                def qk_set(wsb, bsb, dst):
                    xtb = state["xtb"]
                    qkps = sps.tile([128, TCH], f32, tag="S", name="qkps")
                    for cc in range(cc_n):
                        nc.tensor.matmul(
                            qkps[:], wsb[:, cc, :], xtb[:, cc, :],
                            start=(cc == 0), stop=(cc == cc_n - 1))
                    nc.vector.tensor_scalar_add(
                        dst[:, ts(tci, TCH)], qkps[:], bsb[:])

                for wsb, bsb, dst in (
                    (wq01_sb, bq01_sb, Q01),
                    (wk01_sb, bk01_sb, K01),
                    (wq2d_sb, bq2d_sb, Q2),
                    (wk2d_sb, bk2d_sb, K2),
                ):
                    ops_l.append(
                        lambda w=wsb, b=bsb, d=dst: qk_set(w, b, d))

                def v_set(st):
                    xtb = state["xtb"]
                    tt = tci * spk + st
                    vps = ops.tile([128, 195], f32, tag="oT", name="vps")
                    for cc in range(cc_n):
                        nc.tensor.matmul(
                            vps[:], xtb[:, cc, ts(st, 128)], wv_sb[:, cc, :],
                            start=(cc == 0), stop=False)
                    nc.tensor.matmul(vps[:], ones128_sb[:], bv1_sb[:],
                                     start=False, stop=True)
                    nc.vector.tensor_copy(Vp[:, tt, :], vps[:])

                for st in range(spk):
                    ops_l.append(lambda s=st: v_set(s))
                return ops_l

            # ---- output-projection ops for one t-chunk ----
            def proj_ops(tci):
                def do_tile(tt):
                    po1 = ops.tile([128, 512], f32, tag="oT", name="po1")
                    po2 = ops.tile([128, 256], f32, tag="oT", name="po2")
                    for po, cs, cw in ((po1, 0, 512), (po2, 512, 256)):
                        nc.tensor.matmul(po[:], yTa[:, ts(tt, 128)],
                                         wpA_sb[:, ds(cs, cw)],
                                         start=True, stop=False)
                        nc.tensor.matmul(po[:], yT2[:, ts(tt, 128)],
                                         wpB_sb[:, ds(cs, cw)],
                                         start=False, stop=True)
                    pout = xin.tile([128, C], f32, tag="pout", name="pout")
                    nc.vector.tensor_copy(pout[:, 0:512], po1[:])
                    nc.vector.tensor_copy(pout[:, 512:768], po2[:])
                    nc.sync.dma_start(out_d[ts(tt, 128), :], pout[:])

                return [lambda x=(tci * spk + s): do_tile(x)
                        for s in range(spk)]

            # ---- attention ----
            def normalize(oT, h, qc):
                den = wkp.tile([1, TCH], f32, tag="den", name="den")
                nc.vector.tensor_copy(den[:], oT[64:65, :])
                recip = wkp.tile([1, TCH], f32, tag="recip", name="recip")
                nc.vector.reciprocal_approx_fast(out=recip[:], in_=den[:])
                rb = wkp.tile([64, TCH], f32, tag="rb", name="rb")
                nc.gpsimd.partition_broadcast(rb[:], recip[:])
                if h == 0:
                    nc.vector.tensor_mul(yTa[0:64, ts(qc, TCH)], oT[0:64, :],
                                         rb[:])
                elif h == 2:
                    nc.vector.tensor_mul(yT2[0:64, ts(qc, TCH)], oT[0:64, :],
                                         rb[:])
                else:
                    y1t = wkp.tile([64, TCH], bf16, tag="y1t", name="y1t")
                    nc.vector.tensor_mul(y1t[:], oT[0:64, :], rb[:])
                    nc.sync.dma_start(yTa[64:128, ts(qc, TCH)], y1t[:])

            def attention(qc, fillers):
                nkc = (qc + 1) * spk
                q0 = qc * TCH

                def lo_of(kc):
                    m = kc - qc * spk
                    return max(0, 128 * m), m

                # ---- pass 1: heads 0,1 row-tiled ----
                oT0 = ops.tile([65, TCH], f32, tag="oT", name="oT0")
                oT1 = ops.tile([65, TCH], f32, tag="oT", name="oT1")
                s_pend = {}

                def emit_s01(kc):
                    lo, _ = lo_of(kc)
                    S = sps.tile([128, 1024], f32, tag="S", name="S01")
                    nc.tensor.matmul(
                        S[:, lo:TCH],
                        K01[0:64, ts(kc, KCH)], Q01[0:64, ds(q0 + lo,
                                                             TCH - lo)],
                        start=True, stop=True, tile_position=(0, 0))
                    nc.tensor.matmul(
                        S[:, TCH + lo:1024],
                        K01[64:128, ts(kc, KCH)], Q01[64:128, ds(q0 + lo,
                                                                 TCH - lo)],
                        start=True, stop=True, tile_position=(64, 0))
                    s_pend[kc] = S

                emit_s01(0)
                for kc in range(nkc):
                    lo, m = lo_of(kc)
                    S = s_pend.pop(kc)
                    PT = ptp.tile([128, 1024], bf16, tag="PT", name="PT")
                    if lo == 0:
                        nc.scalar.activation(PT[:], S[:], AF.Exp, scale=0.125)
                    else:
                        nc.scalar.activation(PT[:, lo:TCH], S[:, lo:TCH],
                                             AF.Exp, scale=0.125)
                        nc.scalar.activation(PT[:, TCH + lo:1024],
                                             S[:, TCH + lo:1024],
                                             AF.Exp, scale=0.125)
                    if kc + 1 < nkc:
                        emit_s01(kc + 1)
                    if m >= 0:
                        nc.vector.tensor_mul(PT[:, ds(lo, 128)],
                                             PT[:, ds(lo, 128)], tri_sb[:])
                        nc.vector.tensor_mul(PT[:, ds(TCH + lo, 128)],
                                             PT[:, ds(TCH + lo, 128)],
                                             tri_sb[:])
                    nc.tensor.matmul(oT0[:, lo:TCH], Vp[:, kc, 0:65],
                                     PT[:, lo:TCH],
                                     start=(kc == 0), stop=(kc == nkc - 1))
                    nc.tensor.matmul(oT1[:, lo:TCH], Vp[:, kc, 65:130],
                                     PT[:, TCH + lo:1024],
                                     start=(kc == 0), stop=(kc == nkc - 1))
                    if fillers:
                        fillers.pop(0)()
                normalize(oT0, 0, qc)
                normalize(oT1, 1, qc)

                # ---- pass 2: head 2, k-chunk pairs row-tiled ----
                oT2 = ops.tile([65, TCH], f32, tag="oT", name="oT2")
                npair = nkc // 2
                s2_pend = {}

                def emit_s2(kp):
                    kc0, kc1 = 2 * kp, 2 * kp + 1
                    lo0, _ = lo_of(kc0)
                    lo1, _ = lo_of(kc1)
                    S2 = sps.tile([128, 1024], f32, tag="S", name="S2")
                    nc.tensor.matmul(
                        S2[:, lo0:TCH],
                        K2[0:64, ts(kc0, KCH)], Q2[0:64, ds(q0 + lo0,
                                                            TCH - lo0)],
                        start=True, stop=True, tile_position=(0, 0))
                    nc.tensor.matmul(
                        S2[:, TCH + lo1:1024],
                        K2[64:128, ts(kc1, KCH)], Q2[64:128, ds(q0 + lo1,
                                                                TCH - lo1)],
                        start=True, stop=True, tile_position=(64, 0))
                    s2_pend[kp] = S2

                emit_s2(0)
                for kp in range(npair):
                    kc0, kc1 = 2 * kp, 2 * kp + 1
                    lo0, m0 = lo_of(kc0)
                    lo1, m1 = lo_of(kc1)
                    S2 = s2_pend.pop(kp)
                    PT2 = ptp.tile([128, 1024], bf16, tag="PT", name="PT2")
                    if lo0 == 0 and lo1 == 0:
                        nc.scalar.activation(PT2[:], S2[:], AF.Exp,
                                             scale=0.125)
                    else:
                        nc.scalar.activation(PT2[:, lo0:TCH], S2[:, lo0:TCH],
                                             AF.Exp, scale=0.125)
                        nc.scalar.activation(PT2[:, TCH + lo1:1024],
                                             S2[:, TCH + lo1:1024],
                                             AF.Exp, scale=0.125)
                    if kp + 1 < npair:
                        emit_s2(kp + 1)
                    if m0 >= 0:
                        nc.vector.tensor_mul(PT2[:, ds(lo0, 128)],
                                             PT2[:, ds(lo0, 128)], tri_sb[:])
                    if m1 >= 0:
                        nc.vector.tensor_mul(PT2[:, ds(TCH + lo1, 128)],
                                             PT2[:, ds(TCH + lo1, 128)],
                                             tri_sb[:])
                    nc.tensor.matmul(oT2[:, lo0:TCH], Vp[:, kc0, 130:195],
                                     PT2[:, lo0:TCH],
                                     start=(kp == 0), stop=False)
                    nc.tensor.matmul(oT2[:, lo1:TCH], Vp[:, kc1, 130:195],
                                     PT2[:, TCH + lo1:1024],
                                     start=False, stop=(kp == npair - 1))
                    if fillers:
                        fillers.pop(0)()
                normalize(oT2, 2, qc)

            # ---- main schedule: QKV(0) up front, then per-qc attention
            # with next-chunk QKV + prev-chunk proj injected as PE fillers
            for op in qkv_ops(0):
                op()
            for qc in range(nt):
                fillers = []
                if qc + 1 < nt:
                    fillers += qkv_ops(qc + 1)
                if qc >= 1:
                    fillers += proj_ops(qc - 1)
                attention(qc, fillers)
                for op in fillers:
                    op()
            for op in proj_ops(nt - 1):
                op()

    return nc


def make_tri():
    import ml_dtypes
    p = np.arange(128)[:, None]
    j = np.arange(128)[None, :]
    return (j - p >= 0).astype(ml_dtypes.bfloat16)


def core_inputs(c, x, w_attn, b_attn, w_proj, xT_by_batch, tri):
    import ml_dtypes
    f32 = np.float32
    b = c // 4
    heads = [(c % 4) * HPC + i for i in range(HPC)]
    h0, h1, h2 = heads

    def Wq(h):
        return w_attn[:, h * HD:(h + 1) * HD]

    def Wk(h):
        return w_attn[:, C + h * HD:C + (h + 1) * HD]

    def Wv(h):
        return w_attn[:, 2 * C + h * HD:2 * C + (h + 1) * HD]

    def bq(h):
        return b_attn[h * HD:(h + 1) * HD]

    def bk(h):
        return b_attn[C + h * HD:C + (h + 1) * HD]

    def bv(h):
        return b_attn[2 * C + h * HD:2 * C + (h + 1) * HD]

    wv195 = np.zeros((C, 195), f32)
    bv1 = np.zeros((1, 195), f32)
    for i, h in enumerate(heads):
        wv195[:, i * 65:i * 65 + 64] = Wv(h)
        bv1[0, i * 65:i * 65 + 64] = bv(h)
        bv1[0, i * 65 + 64] = 1.0
    return {
        "xT": xT_by_batch[b],
        "wq01": np.ascontiguousarray(np.concatenate([Wq(h0), Wq(h1)], 1)),
        "wk01": np.ascontiguousarray(np.concatenate([Wk(h0), Wk(h1)], 1)),
        "wq2d": np.ascontiguousarray(np.concatenate([Wq(h2), Wq(h2)], 1)),
        "wk2d": np.ascontiguousarray(np.concatenate([Wk(h2), Wk(h2)], 1)),
        "wv": wv195,
        "bv1": bv1.astype(ml_dtypes.bfloat16),
        "bq01": np.ascontiguousarray(
            np.concatenate([bq(h0), bq(h1)])[:, None]),
        "bk01": np.ascontiguousarray(
            np.concatenate([bk(h0), bk(h1)])[:, None]),
        "bq2d": np.ascontiguousarray(
            np.concatenate([bq(h2), bq(h2)])[:, None]),
        "bk2d": np.ascontiguousarray(
            np.concatenate([bk(h2), bk(h2)])[:, None]),
        "wp": np.ascontiguousarray(
            np.concatenate([w_proj[h * HD:(h + 1) * HD, :] for h in heads],
                           0)),
        "tri": tri,
        "ones128": np.ones((1, 128), ml_dtypes.bfloat16),
    }


TRACE = False
LAST_EXEC_NS = None
LAST_RESULTS = None


def kernel(x, w_attn, b_attn, w_proj, b_proj):
    global LAST_EXEC_NS, LAST_RESULTS
    _ensure_axon_hooks_module()
    from concourse.bass_utils import run_bass_kernel_spmd

    x = np.asarray(x, np.float32)
    w_attn = np.asarray(w_attn, np.float32)
    b_attn = np.asarray(b_attn, np.float32)
    w_proj = np.asarray(w_proj, np.float32)
    b_proj = np.asarray(b_proj, np.float32)

    if "nc" not in _cache:
        nc = build_program()
        nc.finalize()
        _cache["nc"] = nc
    nc = _cache["nc"]

    xT_by_batch = [np.ascontiguousarray(x[b].T) for b in range(B)]
    tri = make_tri()
    in_maps = [
        core_inputs(c, x, w_attn, b_attn, w_proj, xT_by_batch, tri)
        for c in range(N_CORES)
    ]
    res = run_bass_kernel_spmd(nc, in_maps, core_ids=list(range(N_CORES)),
                               trace=TRACE)
    LAST_EXEC_NS = res.exec_time_ns
    LAST_RESULTS = res
    out = np.zeros((B, T, C), np.float32)
    for c in range(N_CORES):
        out[c // 4] += np.asarray(res.results[c]["out"], np.float32)
    out += b_proj[None, None, :]
    return out


# revision 10
# speedup vs baseline: 1.6653x; 1.0558x over previous
"""Causal self-attention (GPT-style, B=2, T=4096, C=768, 12 heads) on 8 TRN2
NeuronCores.

Sharding: core c handles batch b = c//4 and heads [3g, 3g+1, 3g+2] with
g = c%4 (data parallel on B x tensor parallel on heads).  Each core computes
its heads' attention output projected through its slice of w_proj; the host
sums the 4 partial [T, C] outputs per batch and adds b_proj.

Device-side formulation (all matmuls bf16, fp32 accumulate):
  - host passes x[b].T so QKV projections contract C on partitions:
      qT/kT  = W.T @ x.T        -> [head_dim(=partitions), T]
      V'     = x @ [Wv|0] + ones-col -> [T(=partitions), 3*65]  (col 64 of
               each 65-block is constant 1 -> PV also yields softmax denoms)
  - scores computed transposed, S^T[k, q] via lhsT=kT, rhs=qT; two heads per
    512-cycle slot via PE row-tiling (K=64 each, concurrent row groups).
  - softmax without max-subtraction (scores are O(5), exp safe in fp32):
      P^T = exp(0.125 * S^T) on ScalarE, PSUM->SBUF bf16, one activate per
      [128, 1024] (both heads / both k-chunks of a slot share it).
  - causal: strictly-masked k-chunks never computed; at boundary tiles the
    score matmul / exp / PV shrink to the valid column subrange and only the
    diagonal [128,128] strip is multiplied by a triangular 0/1 bf16 mask.
  - PV: oT'[65, q] += V'[k,65].T @ P^T[k,q] accumulated over k-chunks; row 64
    is the softmax denominator.  Normalize: approx-reciprocal on DVE,
    partition-broadcast via a stride-0 SBUF->SBUF DMA, multiply -> yT bf16.
  - output projection (packed K=128): out[t,:] = yTa[:,t].T @ wp[h01-rows] +
    yT2[:,t].T @ wp[h2-rows];  h1's normalized slab is DMA-shifted into
    partitions 64..127 of yTa so two heads contract in one matmul.
  - QKV-projection and output-projection work is interleaved into the
    attention loop as PE filler ops so TensorE never idles (keeps the HAM
    clock gate at 2.4 GHz) while ScalarE streams the exps.
"""

import numpy as np

N_CORES = 8
B = 2
T = 4096
C = 768
NH = 12
HD = 64
HPC = 3            # heads per core
TCH = 512          # t / q chunk
KCH = 128          # k chunk
CPART = 128

_cache = {}


def _ensure_axon_hooks_module():
    """Make `from antenv.axon_hooks import ...` importable even on images
    whose antenv package lacks the module (profiling then degrades to a
    no-op instead of crashing run_bass_kernel_spmd(trace=True))."""
    import sys
    import types
    try:
        import antenv.axon_hooks  # noqa: F401
        return
    except Exception:
        pass
    m = types.ModuleType("antenv.axon_hooks")
    m._hook = None

    def _set(h):
        m._hook = h

    def _get():
        return m._hook

    m.set_axon_ntff_profile_hook = _set
    m.get_axon_ntff_profile_hook = _get
    sys.modules["antenv.axon_hooks"] = m


def build_program(t=T):
    """Build the single-core SPMD bass program (same program on all cores,
    per-core data). Returns the un-finalized Bacc."""
    import concourse.mybir as mybir
    import concourse.tile as tile
    from concourse import bacc
    from concourse.bass import ds, ts

    f32 = mybir.dt.float32
    bf16 = mybir.dt.bfloat16
    AF = mybir.ActivationFunctionType

    nt = t // TCH          # number of t/q chunks
    spk = TCH // KCH       # k-chunks per t-chunk (4)
    cc_n = C // CPART      # 6 contraction chunks

    nc = bacc.Bacc("TRN2", target_bir_lowering=False)

    xT = nc.dram_tensor("xT", [C, t], f32, kind="ExternalInput")
    wq01_d = nc.dram_tensor("wq01", [C, 128], f32, kind="ExternalInput")
    wk01_d = nc.dram_tensor("wk01", [C, 128], f32, kind="ExternalInput")
    wqk2_d = nc.dram_tensor("wqk2", [C, 128], f32, kind="ExternalInput")
    wv_d = nc.dram_tensor("wv", [C, 195], f32, kind="ExternalInput")
    bv1_d = nc.dram_tensor("bv1", [1, 195], bf16, kind="ExternalInput")
    bq01_d = nc.dram_tensor("bq01", [128, 1], f32, kind="ExternalInput")
    bk01_d = nc.dram_tensor("bk01", [128, 1], f32, kind="ExternalInput")
    bqk2_d = nc.dram_tensor("bqk2", [128, 1], f32, kind="ExternalInput")
    wp_d = nc.dram_tensor("wp", [HPC * HD, C], f32, kind="ExternalInput")
    tri_d = nc.dram_tensor("tri", [128, 128], bf16, kind="ExternalInput")
    ones128_d = nc.dram_tensor("ones128", [1, 128], bf16, kind="ExternalInput")
    out_d = nc.dram_tensor("out", [t, C], f32, kind="ExternalOutput")

    with tile.TileContext(nc) as tc_:
        with (
            tc_.tile_pool(name="consts", bufs=1) as consts,
            tc_.tile_pool(name="big", bufs=1) as big,
            tc_.tile_pool(name="xin", bufs=2) as xin,
            tc_.tile_pool(name="ptp", bufs=6) as ptp,
            tc_.tile_pool(name="wkp", bufs=3) as wkp,
            tc_.tile_pool(name="sps", bufs=2, space="PSUM") as sps,
            tc_.tile_pool(name="ops", bufs=4, space="PSUM") as ops,
        ):
            # ---- init: weights / biases / masks to SBUF (bf16) ----
            def load_w(nm, dram_ap, shape_f32):
                stage = xin.tile(list(shape_f32), f32, tag=nm + "_st",
                                 name="wstage")
                nc.sync.dma_start(stage[:], dram_ap)
                wsb = consts.tile(list(shape_f32), bf16, name=nm, tag=nm)
                nc.vector.tensor_copy(wsb[:], stage[:])
                return wsb

            rr = "(po pi) m -> pi po m"
            wq01_sb = load_w("wq01_sb", wq01_d[:, :].rearrange(rr, pi=128),
                             [128, cc_n, 128])
            wk01_sb = load_w("wk01_sb", wk01_d[:, :].rearrange(rr, pi=128),
                             [128, cc_n, 128])
            wqk2_sb = load_w("wqk2_sb", wqk2_d[:, :].rearrange(rr, pi=128),
                             [128, cc_n, 128])
            wv_sb = load_w("wv_sb", wv_d[:, :].rearrange(rr, pi=128),
                           [128, cc_n, 195])
            wpA_sb = load_w("wpA_sb", wp_d[0:128, :], [128, C])
            wpB_sb = load_w("wpB_sb", wp_d[128:192, :], [64, C])

            bq01_sb = consts.tile([128, 1], f32)
            bk01_sb = consts.tile([128, 1], f32)
            bqk2_sb = consts.tile([128, 1], f32)
            nc.sync.dma_start(bq01_sb[:], bq01_d[:, :])
            nc.sync.dma_start(bk01_sb[:], bk01_d[:, :])
            nc.sync.dma_start(bqk2_sb[:], bqk2_d[:, :])
            bv1_sb = consts.tile([1, 195], bf16)
            nc.sync.dma_start(bv1_sb[:], bv1_d[:, :])
            ones128_sb = consts.tile([1, 128], bf16)
            nc.sync.dma_start(ones128_sb[:], ones128_d[:, :])
            tri_sb = consts.tile([128, 128], bf16)
            nc.sync.dma_start(tri_sb[:], tri_d[:, :])

            # ---- persistent activations ----
            Q01 = big.tile([128, t], bf16)   # rows 0-63 qT_h0, 64-127 qT_h1
            K01 = big.tile([128, t], bf16)
            Q2 = big.tile([128, t], bf16)    # qT_h2 duplicated on both halves
            K2 = big.tile([128, t], bf16)
            Vp = big.tile([128, t // KCH, 195], bf16)
            yTa = big.tile([128, t], bf16)   # normalized h0 (0:64) | h1
            yT2 = big.tile([64, t], bf16)

            xT_r = xT[:, :].rearrange("(po pi) t -> pi po t", pi=128)

            # ---- QKV projection ops for one t-chunk (list of closures) ----
            def qkv_ops(tci):
                state = {}
                ops_l = []

                def dma_cast():
                    xt32 = xin.tile([128, cc_n, TCH], f32, tag="xt32",
                                    name="xt32")
                    xtb = xin.tile([128, cc_n, TCH], bf16, tag="xtb",
                                   name="xtb")
                    for cc in range(cc_n):
                        nc.sync.dma_start(xt32[:, cc, :],
                                          xT_r[:, cc, ts(tci, TCH)])
                        nc.vector.tensor_copy(xtb[:, cc, :], xt32[:, cc, :])
                    state["xtb"] = xtb
                ops_l.append(dma_cast)

                def qk_set(wsb, bsb, dst):
                    xtb = state["xtb"]
                    qkps = sps.tile([128, TCH], f32, tag="S", name="qkps")
                    for cc in range(cc_n):
                        nc.tensor.matmul(
                            qkps[:], wsb[:, cc, :], xtb[:, cc, :],
                            start=(cc == 0), stop=(cc == cc_n - 1))
                    if dst is None:
                        # packed [qT_h2; kT_h2]: bias-add the aligned halves
                        # into Q2/K2, then DMA-duplicate across halves.
                        nc.vector.tensor_scalar_add(
                            Q2[0:64, ts(tci, TCH)], qkps[0:64, :],
                            bsb[0:64, :])
                        nc.vector.tensor_scalar_add(
                            K2[64:128, ts(tci, TCH)], qkps[64:128, :],
                            bsb[64:128, :])
                        nc.sync.dma_start(Q2[64:128, ts(tci, TCH)],
                                          Q2[0:64, ts(tci, TCH)])
                        nc.sync.dma_start(K2[0:64, ts(tci, TCH)],
                                          K2[64:128, ts(tci, TCH)])
                    else:
                        nc.vector.tensor_scalar_add(
                            dst[:, ts(tci, TCH)], qkps[:], bsb[:])

                for wsb, bsb, dst in (
                    (wq01_sb, bq01_sb, Q01),
                    (wk01_sb, bk01_sb, K01),
                    (wqk2_sb, bqk2_sb, None),
                ):
                    ops_l.append(
                        lambda w=wsb, b=bsb, d=dst: qk_set(w, b, d))

                def v_set(st):
                    xtb = state["xtb"]
                    tt = tci * spk + st
                    vps = ops.tile([128, 195], f32, tag="oT", name="vps")
                    for cc in range(cc_n):
                        nc.tensor.matmul(
                            vps[:], xtb[:, cc, ts(st, 128)], wv_sb[:, cc, :],
                            start=(cc == 0), stop=False)
                    nc.tensor.matmul(vps[:], ones128_sb[:], bv1_sb[:],
                                     start=False, stop=True)
                    nc.vector.tensor_copy(Vp[:, tt, :], vps[:])

                for st in range(spk):
                    ops_l.append(lambda s=st: v_set(s))
                return ops_l

            # ---- output-projection ops for one t-chunk ----
            def proj_ops(tci):
                def do_tile(tt):
                    po1 = ops.tile([128, 512], f32, tag="oT", name="po1")
                    po2 = ops.tile([128, 256], f32, tag="oT", name="po2")
                    for po, cs, cw in ((po1, 0, 512), (po2, 512, 256)):
                        nc.tensor.matmul(po[:], yTa[:, ts(tt, 128)],
                                         wpA_sb[:, ds(cs, cw)],
                                         start=True, stop=False)
                        nc.tensor.matmul(po[:], yT2[:, ts(tt, 128)],
                                         wpB_sb[:, ds(cs, cw)],
                                         start=False, stop=True)
                    pout = xin.tile([128, C], f32, tag="pout", name="pout")
                    nc.vector.tensor_copy(pout[:, 0:512], po1[:])
                    nc.vector.tensor_copy(pout[:, 512:768], po2[:])
                    nc.sync.dma_start(out_d[ts(tt, 128), :], pout[:])

                return [lambda x=(tci * spk + s): do_tile(x)
                        for s in range(spk)]

            # ---- attention ----
            def normalize(oT, h, qc):
                den = wkp.tile([1, TCH], f32, tag="den", name="den")
                nc.vector.tensor_copy(den[:], oT[64:65, :])
                recip = wkp.tile([1, TCH], f32, tag="recip", name="recip")
                nc.vector.reciprocal_approx_fast(out=recip[:], in_=den[:])
                rb = wkp.tile([64, TCH], f32, tag="rb", name="rb")
                nc.gpsimd.partition_broadcast(rb[:], recip[:])
                if h == 0:
                    nc.vector.tensor_mul(yTa[0:64, ts(qc, TCH)], oT[0:64, :],
                                         rb[:])
                elif h == 2:
                    nc.vector.tensor_mul(yT2[0:64, ts(qc, TCH)], oT[0:64, :],
                                         rb[:])
                else:
                    y1t = wkp.tile([64, TCH], bf16, tag="y1t", name="y1t")
                    nc.vector.tensor_mul(y1t[:], oT[0:64, :], rb[:])
                    nc.sync.dma_start(yTa[64:128, ts(qc, TCH)], y1t[:])

            def attention(qc, fillers):
                nkc = (qc + 1) * spk
                q0 = qc * TCH

                def lo_of(kc):
                    m = kc - qc * spk
                    return max(0, 128 * m), m

                # ---- pass 1: heads 0,1 row-tiled ----
                oT0 = ops.tile([65, TCH], f32, tag="oT", name="oT0")
                oT1 = ops.tile([65, TCH], f32, tag="oT", name="oT1")
                s_pend = {}

                def emit_s01(kc):
                    lo, _ = lo_of(kc)
                    S = sps.tile([128, 1024], f32, tag="S", name="S01")
                    nc.tensor.matmul(
                        S[:, lo:TCH],
                        K01[0:64, ts(kc, KCH)], Q01[0:64, ds(q0 + lo,
                                                             TCH - lo)],
                        start=True, stop=True, tile_position=(0, 0))
                    nc.tensor.matmul(
                        S[:, TCH + lo:1024],
                        K01[64:128, ts(kc, KCH)], Q01[64:128, ds(q0 + lo,
                                                                 TCH - lo)],
                        start=True, stop=True, tile_position=(64, 0))
                    s_pend[kc] = S

                emit_s01(0)
                for kc in range(nkc):
                    lo, m = lo_of(kc)
                    S = s_pend.pop(kc)
                    PT = ptp.tile([128, 1024], bf16, tag="PT", name="PT")
                    if lo == 0:
                        nc.scalar.activation(PT[:], S[:], AF.Exp, scale=0.125)
                    else:
                        nc.scalar.activation(PT[:, lo:TCH], S[:, lo:TCH],
                                             AF.Exp, scale=0.125)
                        nc.scalar.activation(PT[:, TCH + lo:1024],
                                             S[:, TCH + lo:1024],
                                             AF.Exp, scale=0.125)
                    if kc + 1 < nkc:
                        emit_s01(kc + 1)
                    if m >= 0:
                        nc.vector.tensor_mul(PT[:, ds(lo, 128)],
                                             PT[:, ds(lo, 128)], tri_sb[:])
                        nc.vector.tensor_mul(PT[:, ds(TCH + lo, 128)],
                                             PT[:, ds(TCH + lo, 128)],
                                             tri_sb[:])
                    nc.tensor.matmul(oT0[:, lo:TCH], Vp[:, kc, 0:65],
                                     PT[:, lo:TCH],
                                     start=(kc == 0), stop=(kc == nkc - 1))
                    nc.tensor.matmul(oT1[:, lo:TCH], Vp[:, kc, 65:130],
                                     PT[:, TCH + lo:1024],
                                     start=(kc == 0), stop=(kc == nkc - 1))
                    if fillers:
                        fillers.pop(0)()
                normalize(oT0, 0, qc)
                normalize(oT1, 1, qc)

                # ---- pass 2: head 2, k-chunk pairs row-tiled ----
                oT2 = ops.tile([65, TCH], f32, tag="oT", name="oT2")
                npair = nkc // 2
                s2_pend = {}

                def emit_s2(kp):
                    kc0, kc1 = 2 * kp, 2 * kp + 1
                    lo0, _ = lo_of(kc0)
                    lo1, _ = lo_of(kc1)
                    S2 = sps.tile([128, 1024], f32, tag="S", name="S2")
                    nc.tensor.matmul(
                        S2[:, lo0:TCH],
                        K2[0:64, ts(kc0, KCH)], Q2[0:64, ds(q0 + lo0,
                                                            TCH - lo0)],
                        start=True, stop=True, tile_position=(0, 0))
                    nc.tensor.matmul(
                        S2[:, TCH + lo1:1024],
                        K2[64:128, ts(kc1, KCH)], Q2[64:128, ds(q0 + lo1,
                                                                TCH - lo1)],
                        start=True, stop=True, tile_position=(64, 0))
                    s2_pend[kp] = S2

                emit_s2(0)
                for kp in range(npair):
                    kc0, kc1 = 2 * kp, 2 * kp + 1
                    lo0, m0 = lo_of(kc0)
                    lo1, m1 = lo_of(kc1)
                    S2 = s2_pend.pop(kp)
                    PT2 = ptp.tile([128, 1024], bf16, tag="PT", name="PT2")
                    if lo0 == 0 and lo1 == 0:
                        nc.scalar.activation(PT2[:], S2[:], AF.Exp,
                                             scale=0.125)
                    else:
                        nc.scalar.activation(PT2[:, lo0:TCH], S2[:, lo0:TCH],
                                             AF.Exp, scale=0.125)
                        nc.scalar.activation(PT2[:, TCH + lo1:1024],
                                             S2[:, TCH + lo1:1024],
                                             AF.Exp, scale=0.125)
                    if kp + 1 < npair:
                        emit_s2(kp + 1)
                    if m0 >= 0:
                        nc.vector.tensor_mul(PT2[:, ds(lo0, 128)],
                                             PT2[:, ds(lo0, 128)], tri_sb[:])
                    if m1 >= 0:
                        nc.vector.tensor_mul(PT2[:, ds(TCH + lo1, 128)],
                                             PT2[:, ds(TCH + lo1, 128)],
                                             tri_sb[:])
                    nc.tensor.matmul(oT2[:, lo0:TCH], Vp[:, kc0, 130:195],
                                     PT2[:, lo0:TCH],
                                     start=(kp == 0), stop=False)
                    nc.tensor.matmul(oT2[:, lo1:TCH], Vp[:, kc1, 130:195],
                                     PT2[:, TCH + lo1:1024],
                                     start=False, stop=(kp == npair - 1))
                    if fillers:
                        fillers.pop(0)()
                normalize(oT2, 2, qc)

            # ---- main schedule: QKV(0) up front, then per-qc attention
            # with next-chunk QKV + prev-chunk proj injected as PE fillers
            for op in qkv_ops(0):
                op()
            for qc in range(nt):
                fillers = []
                if qc + 1 < nt:
                    fillers += qkv_ops(qc + 1)
                if qc >= 1:
                    fillers += proj_ops(qc - 1)
                attention(qc, fillers)
                for op in fillers:
                    op()
            for op in proj_ops(nt - 1):
                op()

    return nc


def make_tri():
    import ml_dtypes
    p = np.arange(128)[:, None]
    j = np.arange(128)[None, :]
    return (j - p >= 0).astype(ml_dtypes.bfloat16)


def core_inputs(c, x, w_attn, b_attn, w_proj, xT_by_batch, tri):
    import ml_dtypes
    f32 = np.float32
    b = c // 4
    heads = [(c % 4) * HPC + i for i in range(HPC)]
    h0, h1, h2 = heads

    def Wq(h):
        return w_attn[:, h * HD:(h + 1) * HD]

    def Wk(h):
        return w_attn[:, C + h * HD:C + (h + 1) * HD]

    def Wv(h):
        return w_attn[:, 2 * C + h * HD:2 * C + (h + 1) * HD]

    def bq(h):
        return b_attn[h * HD:(h + 1) * HD]

    def bk(h):
        return b_attn[C + h * HD:C + (h + 1) * HD]

    def bv(h):
        return b_attn[2 * C + h * HD:2 * C + (h + 1) * HD]

    wv195 = np.zeros((C, 195), f32)
    bv1 = np.zeros((1, 195), f32)
    for i, h in enumerate(heads):
        wv195[:, i * 65:i * 65 + 64] = Wv(h)
        bv1[0, i * 65:i * 65 + 64] = bv(h)
        bv1[0, i * 65 + 64] = 1.0
    return {
        "xT": xT_by_batch[b],
        "wq01": np.ascontiguousarray(np.concatenate([Wq(h0), Wq(h1)], 1)),
        "wk01": np.ascontiguousarray(np.concatenate([Wk(h0), Wk(h1)], 1)),
        "wqk2": np.ascontiguousarray(np.concatenate([Wq(h2), Wk(h2)], 1)),
        "wv": wv195,
        "bv1": bv1.astype(ml_dtypes.bfloat16),
        "bq01": np.ascontiguousarray(
            np.concatenate([bq(h0), bq(h1)])[:, None]),
        "bk01": np.ascontiguousarray(
            np.concatenate([bk(h0), bk(h1)])[:, None]),
        "bqk2": np.ascontiguousarray(
            np.concatenate([bq(h2), bk(h2)])[:, None]),
        "wp": np.ascontiguousarray(
            np.concatenate([w_proj[h * HD:(h + 1) * HD, :] for h in heads],
                           0)),
        "tri": tri,
        "ones128": np.ones((1, 128), ml_dtypes.bfloat16),
    }


TRACE = False
LAST_EXEC_NS = None
LAST_RESULTS = None


def kernel(x, w_attn, b_attn, w_proj, b_proj):
    global LAST_EXEC_NS, LAST_RESULTS
    _ensure_axon_hooks_module()
    from concourse.bass_utils import run_bass_kernel_spmd

    x = np.asarray(x, np.float32)
    w_attn = np.asarray(w_attn, np.float32)
    b_attn = np.asarray(b_attn, np.float32)
    w_proj = np.asarray(w_proj, np.float32)
    b_proj = np.asarray(b_proj, np.float32)

    if "nc" not in _cache:
        nc = build_program()
        nc.finalize()
        _cache["nc"] = nc
    nc = _cache["nc"]

    xT_by_batch = [np.ascontiguousarray(x[b].T) for b in range(B)]
    tri = make_tri()
    in_maps = [
        core_inputs(c, x, w_attn, b_attn, w_proj, xT_by_batch, tri)
        for c in range(N_CORES)
    ]
    res = run_bass_kernel_spmd(nc, in_maps, core_ids=list(range(N_CORES)),
                               trace=TRACE)
    LAST_EXEC_NS = res.exec_time_ns
    LAST_RESULTS = res
    out = np.zeros((B, T, C), np.float32)
    for c in range(N_CORES):
        out[c // 4] += np.asarray(res.results[c]["out"], np.float32)
    out += b_proj[None, None, :]
    return out


# revision 11
# speedup vs baseline: 1.6668x; 1.0009x over previous
"""Causal self-attention (GPT-style, B=2, T=4096, C=768, 12 heads) on 8 TRN2
NeuronCores.

Sharding: core c handles batch b = c//4 and heads [3g, 3g+1, 3g+2] with
g = c%4 (data parallel on B x tensor parallel on heads).  Each core computes
its heads' attention output projected through its slice of w_proj; the host
sums the 4 partial [T, C] outputs per batch and adds b_proj.

Device-side formulation (all matmuls bf16, fp32 accumulate):
  - host passes x[b].T so QKV projections contract C on partitions:
      qT/kT  = W.T @ x.T        -> [head_dim(=partitions), T]
      V'     = x @ [Wv|0] + ones-col -> [T(=partitions), 3*65]  (col 64 of
               each 65-block is constant 1 -> PV also yields softmax denoms)
  - scores computed transposed, S^T[k, q] via lhsT=kT, rhs=qT; two heads per
    512-cycle slot via PE row-tiling (K=64 each, concurrent row groups).
  - softmax without max-subtraction (scores are O(5), exp safe in fp32):
      P^T = exp(0.125 * S^T) on ScalarE, PSUM->SBUF bf16, one activate per
      [128, 1024] (both heads / both k-chunks of a slot share it).
  - causal: strictly-masked k-chunks never computed; at boundary tiles the
    score matmul / exp / PV shrink to the valid column subrange and only the
    diagonal [128,128] strip is multiplied by a triangular 0/1 bf16 mask.
  - PV: oT'[65, q] += V'[k,65].T @ P^T[k,q] accumulated over k-chunks; row 64
    is the softmax denominator.  Normalize: approx-reciprocal on DVE,
    partition-broadcast via a stride-0 SBUF->SBUF DMA, multiply -> yT bf16.
  - output projection (packed K=128): out[t,:] = yTa[:,t].T @ wp[h01-rows] +
    yT2[:,t].T @ wp[h2-rows];  h1's normalized slab is DMA-shifted into
    partitions 64..127 of yTa so two heads contract in one matmul.
  - QKV-projection and output-projection work is interleaved into the
    attention loop as PE filler ops so TensorE never idles (keeps the HAM
    clock gate at 2.4 GHz) while ScalarE streams the exps.
"""

import numpy as np

N_CORES = 8
B = 2
T = 4096
C = 768
NH = 12
HD = 64
HPC = 3            # heads per core
TCH = 512          # t / q chunk
KCH = 128          # k chunk
CPART = 128

_cache = {}


def _ensure_axon_hooks_module():
    """Make `from antenv.axon_hooks import ...` importable even on images
    whose antenv package lacks the module (profiling then degrades to a
    no-op instead of crashing run_bass_kernel_spmd(trace=True))."""
    import sys
    import types
    try:
        import antenv.axon_hooks  # noqa: F401
        return
    except Exception:
        pass
    m = types.ModuleType("antenv.axon_hooks")
    m._hook = None

    def _set(h):
        m._hook = h

    def _get():
        return m._hook

    m.set_axon_ntff_profile_hook = _set
    m.get_axon_ntff_profile_hook = _get
    sys.modules["antenv.axon_hooks"] = m


def build_program(t=T):
    """Build the single-core SPMD bass program (same program on all cores,
    per-core data). Returns the un-finalized Bacc."""
    import concourse.mybir as mybir
    import concourse.tile as tile
    from concourse import bacc
    from concourse.bass import ds, ts

    f32 = mybir.dt.float32
    bf16 = mybir.dt.bfloat16
    AF = mybir.ActivationFunctionType

    nt = t // TCH          # number of t/q chunks
    spk = TCH // KCH       # k-chunks per t-chunk (4)
    cc_n = C // CPART      # 6 contraction chunks

    nc = bacc.Bacc("TRN2", target_bir_lowering=False)

    xT = nc.dram_tensor("xT", [C, t], bf16, kind="ExternalInput")
    wq01_d = nc.dram_tensor("wq01", [C, 128], bf16, kind="ExternalInput")
    wk01_d = nc.dram_tensor("wk01", [C, 128], bf16, kind="ExternalInput")
    wqk2_d = nc.dram_tensor("wqk2", [C, 128], bf16, kind="ExternalInput")
    wv_d = nc.dram_tensor("wv", [C, 195], bf16, kind="ExternalInput")
    bv1_d = nc.dram_tensor("bv1", [1, 195], bf16, kind="ExternalInput")
    bq01_d = nc.dram_tensor("bq01", [128, 1], f32, kind="ExternalInput")
    bk01_d = nc.dram_tensor("bk01", [128, 1], f32, kind="ExternalInput")
    bqk2_d = nc.dram_tensor("bqk2", [128, 1], f32, kind="ExternalInput")
    wp_d = nc.dram_tensor("wp", [HPC * HD, C], bf16, kind="ExternalInput")
    tri_d = nc.dram_tensor("tri", [128, 128], bf16, kind="ExternalInput")
    ones128_d = nc.dram_tensor("ones128", [1, 128], bf16, kind="ExternalInput")
    out_d = nc.dram_tensor("out", [t, C], f32, kind="ExternalOutput")

    with tile.TileContext(nc) as tc_:
        with (
            tc_.tile_pool(name="consts", bufs=1) as consts,
            tc_.tile_pool(name="big", bufs=1) as big,
            tc_.tile_pool(name="xin", bufs=2) as xin,
            tc_.tile_pool(name="ptp", bufs=6) as ptp,
            tc_.tile_pool(name="wkp", bufs=3) as wkp,
            tc_.tile_pool(name="sps", bufs=2, space="PSUM") as sps,
            tc_.tile_pool(name="ops", bufs=4, space="PSUM") as ops,
        ):
            # ---- init: weights / biases / masks to SBUF (bf16) ----
            def load_w(nm, dram_ap, shape_bf):
                wsb = consts.tile(list(shape_bf), bf16, name=nm, tag=nm)
                nc.sync.dma_start(wsb[:], dram_ap)
                return wsb

            rr = "(po pi) m -> pi po m"
            wq01_sb = load_w("wq01_sb", wq01_d[:, :].rearrange(rr, pi=128),
                             [128, cc_n, 128])
            wk01_sb = load_w("wk01_sb", wk01_d[:, :].rearrange(rr, pi=128),
                             [128, cc_n, 128])
            wqk2_sb = load_w("wqk2_sb", wqk2_d[:, :].rearrange(rr, pi=128),
                             [128, cc_n, 128])
            wv_sb = load_w("wv_sb", wv_d[:, :].rearrange(rr, pi=128),
                           [128, cc_n, 195])
            wpA_sb = load_w("wpA_sb", wp_d[0:128, :], [128, C])
            wpB_sb = load_w("wpB_sb", wp_d[128:192, :], [64, C])

            bq01_sb = consts.tile([128, 1], f32)
            bk01_sb = consts.tile([128, 1], f32)
            bqk2_sb = consts.tile([128, 1], f32)
            nc.sync.dma_start(bq01_sb[:], bq01_d[:, :])
            nc.sync.dma_start(bk01_sb[:], bk01_d[:, :])
            nc.sync.dma_start(bqk2_sb[:], bqk2_d[:, :])
            bv1_sb = consts.tile([1, 195], bf16)
            nc.sync.dma_start(bv1_sb[:], bv1_d[:, :])
            ones128_sb = consts.tile([1, 128], bf16)
            nc.sync.dma_start(ones128_sb[:], ones128_d[:, :])
            tri_sb = consts.tile([128, 128], bf16)
            nc.sync.dma_start(tri_sb[:], tri_d[:, :])

            # ---- persistent activations ----
            Q01 = big.tile([128, t], bf16)   # rows 0-63 qT_h0, 64-127 qT_h1
            K01 = big.tile([128, t], bf16)
            Q2 = big.tile([128, t], bf16)    # qT_h2 duplicated on both halves
            K2 = big.tile([128, t], bf16)
            Vp = big.tile([128, t // KCH, 195], bf16)
            yTa = big.tile([128, t], bf16)   # normalized h0 (0:64) | h1
            yT2 = big.tile([64, t], bf16)

            xT_r = xT[:, :].rearrange("(po pi) t -> pi po t", pi=128)

            # ---- QKV projection ops for one t-chunk (list of closures) ----
            def qkv_ops(tci):
                state = {}
                ops_l = []

                def dma_cast():
                    xtb = xin.tile([128, cc_n, TCH], bf16, tag="xtb",
                                   name="xtb")
                    for cc in range(cc_n):
                        nc.sync.dma_start(xtb[:, cc, :],
                                          xT_r[:, cc, ts(tci, TCH)])
                    state["xtb"] = xtb
                ops_l.append(dma_cast)

                def qk_set(wsb, bsb, dst):
                    xtb = state["xtb"]
                    qkps = sps.tile([128, TCH], f32, tag="S", name="qkps")
                    for cc in range(cc_n):
                        nc.tensor.matmul(
                            qkps[:], wsb[:, cc, :], xtb[:, cc, :],
                            start=(cc == 0), stop=(cc == cc_n - 1))
                    if dst is None:
                        # packed [qT_h2; kT_h2]: bias-add the aligned halves
                        # into Q2/K2, then DMA-duplicate across halves.
                        nc.vector.tensor_scalar_add(
                            Q2[0:64, ts(tci, TCH)], qkps[0:64, :],
                            bsb[0:64, :])
                        nc.vector.tensor_scalar_add(
                            K2[64:128, ts(tci, TCH)], qkps[64:128, :],
                            bsb[64:128, :])
                        nc.sync.dma_start(Q2[64:128, ts(tci, TCH)],
                                          Q2[0:64, ts(tci, TCH)])
                        nc.sync.dma_start(K2[0:64, ts(tci, TCH)],
                                          K2[64:128, ts(tci, TCH)])
                    else:
                        nc.vector.tensor_scalar_add(
                            dst[:, ts(tci, TCH)], qkps[:], bsb[:])

                for wsb, bsb, dst in (
                    (wq01_sb, bq01_sb, Q01),
                    (wk01_sb, bk01_sb, K01),
                    (wqk2_sb, bqk2_sb, None),
                ):
                    ops_l.append(
                        lambda w=wsb, b=bsb, d=dst: qk_set(w, b, d))

                def v_set(st):
                    xtb = state["xtb"]
                    tt = tci * spk + st
                    vps = ops.tile([128, 195], f32, tag="oT", name="vps")
                    for cc in range(cc_n):
                        nc.tensor.matmul(
                            vps[:], xtb[:, cc, ts(st, 128)], wv_sb[:, cc, :],
                            start=(cc == 0), stop=False)
                    nc.tensor.matmul(vps[:], ones128_sb[:], bv1_sb[:],
                                     start=False, stop=True)
                    nc.vector.tensor_copy(Vp[:, tt, :], vps[:])

                for st in range(spk):
                    ops_l.append(lambda s=st: v_set(s))
                return ops_l

            # ---- output-projection ops for one t-chunk ----
            def proj_ops(tci):
                def do_tile(tt):
                    po1 = ops.tile([128, 512], f32, tag="oT", name="po1")
                    po2 = ops.tile([128, 256], f32, tag="oT", name="po2")
                    for po, cs, cw in ((po1, 0, 512), (po2, 512, 256)):
                        nc.tensor.matmul(po[:], yTa[:, ts(tt, 128)],
                                         wpA_sb[:, ds(cs, cw)],
                                         start=True, stop=False)
                        nc.tensor.matmul(po[:], yT2[:, ts(tt, 128)],
                                         wpB_sb[:, ds(cs, cw)],
                                         start=False, stop=True)
                    pout = xin.tile([128, C], f32, tag="pout", name="pout")
                    nc.vector.tensor_copy(pout[:, 0:512], po1[:])
                    nc.vector.tensor_copy(pout[:, 512:768], po2[:])
                    nc.sync.dma_start(out_d[ts(tt, 128), :], pout[:])

                return [lambda x=(tci * spk + s): do_tile(x)
                        for s in range(spk)]

            # ---- attention ----
            def normalize(oT, h, qc):
                den = wkp.tile([1, TCH], f32, tag="den", name="den")
                nc.vector.tensor_copy(den[:], oT[64:65, :])
                recip = wkp.tile([1, TCH], f32, tag="recip", name="recip")
                nc.vector.reciprocal_approx_fast(out=recip[:], in_=den[:])
                rb = wkp.tile([64, TCH], f32, tag="rb", name="rb")
                nc.gpsimd.partition_broadcast(rb[:], recip[:])
                if h == 0:
                    nc.vector.tensor_mul(yTa[0:64, ts(qc, TCH)], oT[0:64, :],
                                         rb[:])
                elif h == 2:
                    nc.vector.tensor_mul(yT2[0:64, ts(qc, TCH)], oT[0:64, :],
                                         rb[:])
                else:
                    y1t = wkp.tile([64, TCH], bf16, tag="y1t", name="y1t")
                    nc.vector.tensor_mul(y1t[:], oT[0:64, :], rb[:])
                    nc.sync.dma_start(yTa[64:128, ts(qc, TCH)], y1t[:])

            def attention(qc, fillers):
                nkc = (qc + 1) * spk
                q0 = qc * TCH

                def lo_of(kc):
                    m = kc - qc * spk
                    return max(0, 128 * m), m

                # ---- pass 1: heads 0,1 row-tiled ----
                oT0 = ops.tile([65, TCH], f32, tag="oT", name="oT0")
                oT1 = ops.tile([65, TCH], f32, tag="oT", name="oT1")
                s_pend = {}

                def emit_s01(kc):
                    lo, _ = lo_of(kc)
                    S = sps.tile([128, 1024], f32, tag="S", name="S01")
                    nc.tensor.matmul(
                        S[:, lo:TCH],
                        K01[0:64, ts(kc, KCH)], Q01[0:64, ds(q0 + lo,
                                                             TCH - lo)],
                        start=True, stop=True, tile_position=(0, 0))
                    nc.tensor.matmul(
                        S[:, TCH + lo:1024],
                        K01[64:128, ts(kc, KCH)], Q01[64:128, ds(q0 + lo,
                                                                 TCH - lo)],
                        start=True, stop=True, tile_position=(64, 0))
                    s_pend[kc] = S

                emit_s01(0)
                for kc in range(nkc):
                    lo, m = lo_of(kc)
                    S = s_pend.pop(kc)
                    PT = ptp.tile([128, 1024], bf16, tag="PT", name="PT")
                    if lo == 0:
                        nc.scalar.activation(PT[:], S[:], AF.Exp, scale=0.125)
                    else:
                        nc.scalar.activation(PT[:, lo:TCH], S[:, lo:TCH],
                                             AF.Exp, scale=0.125)
                        nc.scalar.activation(PT[:, TCH + lo:1024],
                                             S[:, TCH + lo:1024],
                                             AF.Exp, scale=0.125)
                    if kc + 1 < nkc:
                        emit_s01(kc + 1)
                    if m >= 0:
                        nc.vector.tensor_mul(PT[:, ds(lo, 128)],
                                             PT[:, ds(lo, 128)], tri_sb[:])
                        nc.vector.tensor_mul(PT[:, ds(TCH + lo, 128)],
                                             PT[:, ds(TCH + lo, 128)],
                                             tri_sb[:])
                    nc.tensor.matmul(oT0[:, lo:TCH], Vp[:, kc, 0:65],
                                     PT[:, lo:TCH],
                                     start=(kc == 0), stop=(kc == nkc - 1))
                    nc.tensor.matmul(oT1[:, lo:TCH], Vp[:, kc, 65:130],
                                     PT[:, TCH + lo:1024],
                                     start=(kc == 0), stop=(kc == nkc - 1))
                    if fillers:
                        fillers.pop(0)()
                normalize(oT0, 0, qc)
                normalize(oT1, 1, qc)

                # ---- pass 2: head 2, k-chunk pairs row-tiled ----
                oT2 = ops.tile([65, TCH], f32, tag="oT", name="oT2")
                npair = nkc // 2
                s2_pend = {}

                def emit_s2(kp):
                    kc0, kc1 = 2 * kp, 2 * kp + 1
                    lo0, _ = lo_of(kc0)
                    lo1, _ = lo_of(kc1)
                    S2 = sps.tile([128, 1024], f32, tag="S", name="S2")
                    nc.tensor.matmul(
                        S2[:, lo0:TCH],
                        K2[0:64, ts(kc0, KCH)], Q2[0:64, ds(q0 + lo0,
                                                            TCH - lo0)],
                        start=True, stop=True, tile_position=(0, 0))
                    nc.tensor.matmul(
                        S2[:, TCH + lo1:1024],
                        K2[64:128, ts(kc1, KCH)], Q2[64:128, ds(q0 + lo1,
                                                                TCH - lo1)],
                        start=True, stop=True, tile_position=(64, 0))
                    s2_pend[kp] = S2

                emit_s2(0)
                for kp in range(npair):
                    kc0, kc1 = 2 * kp, 2 * kp + 1
                    lo0, m0 = lo_of(kc0)
                    lo1, m1 = lo_of(kc1)
                    S2 = s2_pend.pop(kp)
                    PT2 = ptp.tile([128, 1024], bf16, tag="PT", name="PT2")
                    if lo0 == 0 and lo1 == 0:
                        nc.scalar.activation(PT2[:], S2[:], AF.Exp,
                                             scale=0.125)
                    else:
                        nc.scalar.activation(PT2[:, lo0:TCH], S2[:, lo0:TCH],
                                             AF.Exp, scale=0.125)
                        nc.scalar.activation(PT2[:, TCH + lo1:1024],
                                             S2[:, TCH + lo1:1024],
                                             AF.Exp, scale=0.125)
                    if kp + 1 < npair:
                        emit_s2(kp + 1)
                    if m0 >= 0:
                        nc.vector.tensor_mul(PT2[:, ds(lo0, 128)],
                                             PT2[:, ds(lo0, 128)], tri_sb[:])
                    if m1 >= 0:
                        nc.vector.tensor_mul(PT2[:, ds(TCH + lo1, 128)],
                                             PT2[:, ds(TCH + lo1, 128)],
                                             tri_sb[:])
                    nc.tensor.matmul(oT2[:, lo0:TCH], Vp[:, kc0, 130:195],
                                     PT2[:, lo0:TCH],
                                     start=(kp == 0), stop=False)
                    nc.tensor.matmul(oT2[:, lo1:TCH], Vp[:, kc1, 130:195],
                                     PT2[:, TCH + lo1:1024],
                                     start=False, stop=(kp == npair - 1))
                    if fillers:
                        fillers.pop(0)()
                normalize(oT2, 2, qc)

            # ---- main schedule: QKV(0) up front, then per-qc attention
            # with next-chunk QKV + prev-chunk proj injected as PE fillers
            for op in qkv_ops(0):
                op()
            for qc in range(nt):
                fillers = []
                if qc + 1 < nt:
                    fillers += qkv_ops(qc + 1)
                if qc >= 1:
                    fillers += proj_ops(qc - 1)
                attention(qc, fillers)
                for op in fillers:
                    op()
            for op in proj_ops(nt - 1):
                op()

    return nc


def make_tri():
    import ml_dtypes
    p = np.arange(128)[:, None]
    j = np.arange(128)[None, :]
    return (j - p >= 0).astype(ml_dtypes.bfloat16)


def core_inputs(c, x, w_attn, b_attn, w_proj, xT_by_batch, tri):
    import ml_dtypes
    f32 = np.float32
    b = c // 4
    heads = [(c % 4) * HPC + i for i in range(HPC)]
    h0, h1, h2 = heads

    def Wq(h):
        return w_attn[:, h * HD:(h + 1) * HD]

    def Wk(h):
        return w_attn[:, C + h * HD:C + (h + 1) * HD]

    def Wv(h):
        return w_attn[:, 2 * C + h * HD:2 * C + (h + 1) * HD]

    def bq(h):
        return b_attn[h * HD:(h + 1) * HD]

    def bk(h):
        return b_attn[C + h * HD:C + (h + 1) * HD]

    def bv(h):
        return b_attn[2 * C + h * HD:2 * C + (h + 1) * HD]

    wv195 = np.zeros((C, 195), f32)
    bv1 = np.zeros((1, 195), f32)
    for i, h in enumerate(heads):
        wv195[:, i * 65:i * 65 + 64] = Wv(h)
        bv1[0, i * 65:i * 65 + 64] = bv(h)
        bv1[0, i * 65 + 64] = 1.0
    bf = ml_dtypes.bfloat16
    return {
        "xT": xT_by_batch[b],
        "wq01": np.ascontiguousarray(
            np.concatenate([Wq(h0), Wq(h1)], 1)).astype(bf),
        "wk01": np.ascontiguousarray(
            np.concatenate([Wk(h0), Wk(h1)], 1)).astype(bf),
        "wqk2": np.ascontiguousarray(
            np.concatenate([Wq(h2), Wk(h2)], 1)).astype(bf),
        "wv": wv195.astype(bf),
        "bv1": bv1.astype(ml_dtypes.bfloat16),
        "bq01": np.ascontiguousarray(
            np.concatenate([bq(h0), bq(h1)])[:, None]),
        "bk01": np.ascontiguousarray(
            np.concatenate([bk(h0), bk(h1)])[:, None]),
        "bqk2": np.ascontiguousarray(
            np.concatenate([bq(h2), bk(h2)])[:, None]),
        "wp": np.ascontiguousarray(
            np.concatenate([w_proj[h * HD:(h + 1) * HD, :] for h in heads],
                           0)).astype(bf),
        "tri": tri,
        "ones128": np.ones((1, 128), ml_dtypes.bfloat16),
    }


TRACE = False
LAST_EXEC_NS = None
LAST_RESULTS = None


def kernel(x, w_attn, b_attn, w_proj, b_proj):
    global LAST_EXEC_NS, LAST_RESULTS
    _ensure_axon_hooks_module()
    from concourse.bass_utils import run_bass_kernel_spmd

    x = np.asarray(x, np.float32)
    w_attn = np.asarray(w_attn, np.float32)
    b_attn = np.asarray(b_attn, np.float32)
    w_proj = np.asarray(w_proj, np.float32)
    b_proj = np.asarray(b_proj, np.float32)

    if "nc" not in _cache:
        nc = build_program()
        nc.finalize()
        _cache["nc"] = nc
    nc = _cache["nc"]

    import ml_dtypes
    xT_by_batch = [np.ascontiguousarray(x[b].T).astype(ml_dtypes.bfloat16)
                   for b in range(B)]
    tri = make_tri()
    in_maps = [
        core_inputs(c, x, w_attn, b_attn, w_proj, xT_by_batch, tri)
        for c in range(N_CORES)
    ]
    res = run_bass_kernel_spmd(nc, in_maps, core_ids=list(range(N_CORES)),
                               trace=TRACE)
    LAST_EXEC_NS = res.exec_time_ns
    LAST_RESULTS = res
    out = np.zeros((B, T, C), np.float32)
    for c in range(N_CORES):
        out[c // 4] += np.asarray(res.results[c]["out"], np.float32)
    out += b_proj[None, None, :]
    return out


# revision 12
# speedup vs baseline: 1.7560x; 1.0535x over previous
"""Causal self-attention (GPT-style, B=2, T=4096, C=768, 12 heads) on 8 TRN2
NeuronCores.

Sharding: core c handles batch b = c//4 and heads [3g, 3g+1, 3g+2] with
g = c%4 (data parallel on B x tensor parallel on heads).  Each core computes
its heads' attention output projected through its slice of w_proj; the host
sums the 4 partial [T, C] outputs per batch and adds b_proj.

Device-side formulation (all matmuls bf16, fp32 accumulate):
  - host passes x[b].T so QKV projections contract C on partitions:
      qT/kT  = W.T @ x.T        -> [head_dim(=partitions), T]
      V'     = x @ [Wv|0] + ones-col -> [T(=partitions), 3*65]  (col 64 of
               each 65-block is constant 1 -> PV also yields softmax denoms)
  - scores computed transposed, S^T[k, q] via lhsT=kT, rhs=qT; two heads per
    512-cycle slot via PE row-tiling (K=64 each, concurrent row groups).
  - softmax without max-subtraction (scores are O(5), exp safe in fp32):
      P^T = exp(0.125 * S^T) on ScalarE, PSUM->SBUF bf16, one activate per
      [128, 1024] (both heads / both k-chunks of a slot share it).
  - causal: strictly-masked k-chunks never computed; at boundary tiles the
    score matmul / exp / PV shrink to the valid column subrange and only the
    diagonal [128,128] strip is multiplied by a triangular 0/1 bf16 mask.
  - PV: oT'[65, q] += V'[k,65].T @ P^T[k,q] accumulated over k-chunks; row 64
    is the softmax denominator.  Normalize: approx-reciprocal on DVE,
    partition-broadcast via a stride-0 SBUF->SBUF DMA, multiply -> yT bf16.
  - output projection (packed K=128): out[t,:] = yTa[:,t].T @ wp[h01-rows] +
    yT2[:,t].T @ wp[h2-rows];  h1's normalized slab is DMA-shifted into
    partitions 64..127 of yTa so two heads contract in one matmul.
  - QKV-projection and output-projection work is interleaved into the
    attention loop as PE filler ops so TensorE never idles (keeps the HAM
    clock gate at 2.4 GHz) while ScalarE streams the exps.
"""

import numpy as np

N_CORES = 8
B = 2
T = 4096
C = 768
NH = 12
HD = 64
HPC = 3            # heads per core
TCH = 512          # t / q chunk
KCH = 128          # k chunk
CPART = 128

_cache = {}


def _ensure_axon_hooks_module():
    """Make `from antenv.axon_hooks import ...` importable even on images
    whose antenv package lacks the module (profiling then degrades to a
    no-op instead of crashing run_bass_kernel_spmd(trace=True))."""
    import sys
    import types
    try:
        import antenv.axon_hooks  # noqa: F401
        return
    except Exception:
        pass
    m = types.ModuleType("antenv.axon_hooks")
    m._hook = None

    def _set(h):
        m._hook = h

    def _get():
        return m._hook

    m.set_axon_ntff_profile_hook = _set
    m.get_axon_ntff_profile_hook = _get
    sys.modules["antenv.axon_hooks"] = m


def build_program(t=T):
    """Build the single-core SPMD bass program (same program on all cores,
    per-core data). Returns the un-finalized Bacc."""
    import concourse.mybir as mybir
    import concourse.tile as tile
    from concourse import bacc
    from concourse.bass import ds, ts

    f32 = mybir.dt.float32
    bf16 = mybir.dt.bfloat16
    AF = mybir.ActivationFunctionType

    nt = t // TCH          # number of t/q chunks
    spk = TCH // KCH       # k-chunks per t-chunk (4)
    cc_n = C // CPART      # 6 contraction chunks

    nc = bacc.Bacc("TRN2", target_bir_lowering=False)

    xT = nc.dram_tensor("xT", [C, t], bf16, kind="ExternalInput")
    wq01_d = nc.dram_tensor("wq01", [128, (C // CPART) * 128], bf16, kind="ExternalInput")
    wk01_d = nc.dram_tensor("wk01", [128, (C // CPART) * 128], bf16, kind="ExternalInput")
    wqk2_d = nc.dram_tensor("wqk2", [128, (C // CPART) * 128], bf16, kind="ExternalInput")
    wv_d = nc.dram_tensor("wv", [128, (C // CPART) * 195], bf16, kind="ExternalInput")
    bv1_d = nc.dram_tensor("bv1", [1, 195], bf16, kind="ExternalInput")
    bq01_d = nc.dram_tensor("bq01", [128, 1], f32, kind="ExternalInput")
    bk01_d = nc.dram_tensor("bk01", [128, 1], f32, kind="ExternalInput")
    bqk2_d = nc.dram_tensor("bqk2", [128, 1], f32, kind="ExternalInput")
    wp_d = nc.dram_tensor("wp", [HPC * HD, C], bf16, kind="ExternalInput")
    tri_d = nc.dram_tensor("tri", [128, 128], bf16, kind="ExternalInput")
    ones128_d = nc.dram_tensor("ones128", [1, 128], bf16, kind="ExternalInput")
    out_d = nc.dram_tensor("out", [t, C], f32, kind="ExternalOutput")

    with tile.TileContext(nc) as tc_:
        with (
            tc_.tile_pool(name="consts", bufs=1) as consts,
            tc_.tile_pool(name="big", bufs=1) as big,
            tc_.tile_pool(name="xin", bufs=2) as xin,
            tc_.tile_pool(name="ptp", bufs=6) as ptp,
            tc_.tile_pool(name="wkp", bufs=3) as wkp,
            tc_.tile_pool(name="sps", bufs=2, space="PSUM") as sps,
            tc_.tile_pool(name="ops", bufs=4, space="PSUM") as ops,
        ):
            # ---- init: weights / biases / masks to SBUF (bf16) ----
            def load_w(nm, dram_ap, shape_bf):
                wsb = consts.tile(list(shape_bf), bf16, name=nm, tag=nm)
                nc.sync.dma_start(wsb[:], dram_ap)
                return wsb

            wq01_sb = load_w("wq01_sb", wq01_d[:, :], [128, cc_n, 128])
            wk01_sb = load_w("wk01_sb", wk01_d[:, :], [128, cc_n, 128])
            wqk2_sb = load_w("wqk2_sb", wqk2_d[:, :], [128, cc_n, 128])
            wv_sb = load_w("wv_sb", wv_d[:, :], [128, cc_n, 195])
            wpA_sb = load_w("wpA_sb", wp_d[0:128, :], [128, C])
            wpB_sb = load_w("wpB_sb", wp_d[128:192, :], [64, C])

            bq01_sb = consts.tile([128, 1], f32)
            bk01_sb = consts.tile([128, 1], f32)
            bqk2_sb = consts.tile([128, 1], f32)
            nc.sync.dma_start(bq01_sb[:], bq01_d[:, :])
            nc.sync.dma_start(bk01_sb[:], bk01_d[:, :])
            nc.sync.dma_start(bqk2_sb[:], bqk2_d[:, :])
            bv1_sb = consts.tile([1, 195], bf16)
            nc.sync.dma_start(bv1_sb[:], bv1_d[:, :])
            ones128_sb = consts.tile([1, 128], bf16)
            nc.sync.dma_start(ones128_sb[:], ones128_d[:, :])
            tri_sb = consts.tile([128, 128], bf16)
            nc.sync.dma_start(tri_sb[:], tri_d[:, :])

            # ---- persistent activations ----
            Q01 = big.tile([128, t], bf16)   # rows 0-63 qT_h0, 64-127 qT_h1
            K01 = big.tile([128, t], bf16)
            Q2 = big.tile([128, t], bf16)    # qT_h2 duplicated on both halves
            K2 = big.tile([128, t], bf16)
            Vp = big.tile([128, t // KCH, 195], bf16)
            yTa = big.tile([128, t], bf16)   # normalized h0 (0:64) | h1
            yT2 = big.tile([64, t], bf16)

            xT_r = xT[:, :].rearrange("(po pi) t -> pi po t", pi=128)

            # ---- QKV projection ops for one t-chunk (list of closures) ----
            def qkv_ops(tci):
                state = {}
                ops_l = []

                def dma_cast():
                    xtb = xin.tile([128, cc_n, TCH], bf16, tag="xtb",
                                   name="xtb")
                    for cc in range(cc_n):
                        nc.sync.dma_start(xtb[:, cc, :],
                                          xT_r[:, cc, ts(tci, TCH)])
                    state["xtb"] = xtb
                ops_l.append(dma_cast)

                def qk_set(wsb, bsb, dst):
                    xtb = state["xtb"]
                    qkps = sps.tile([128, TCH], f32, tag="S", name="qkps")
                    for cc in range(cc_n):
                        nc.tensor.matmul(
                            qkps[:], wsb[:, cc, :], xtb[:, cc, :],
                            start=(cc == 0), stop=(cc == cc_n - 1))
                    if dst is None:
                        # packed [qT_h2; kT_h2]: bias-add the aligned halves
                        # into Q2/K2, then DMA-duplicate across halves.
                        nc.vector.tensor_scalar_add(
                            Q2[0:64, ts(tci, TCH)], qkps[0:64, :],
                            bsb[0:64, :])
                        nc.vector.tensor_scalar_add(
                            K2[64:128, ts(tci, TCH)], qkps[64:128, :],
                            bsb[64:128, :])
                        nc.sync.dma_start(Q2[64:128, ts(tci, TCH)],
                                          Q2[0:64, ts(tci, TCH)])
                        nc.sync.dma_start(K2[0:64, ts(tci, TCH)],
                                          K2[64:128, ts(tci, TCH)])
                    else:
                        nc.vector.tensor_scalar_add(
                            dst[:, ts(tci, TCH)], qkps[:], bsb[:])

                for wsb, bsb, dst in (
                    (wq01_sb, bq01_sb, Q01),
                    (wk01_sb, bk01_sb, K01),
                    (wqk2_sb, bqk2_sb, None),
                ):
                    ops_l.append(
                        lambda w=wsb, b=bsb, d=dst: qk_set(w, b, d))

                def v_set(st):
                    xtb = state["xtb"]
                    tt = tci * spk + st
                    vps = ops.tile([128, 195], f32, tag="oT", name="vps")
                    for cc in range(cc_n):
                        nc.tensor.matmul(
                            vps[:], xtb[:, cc, ts(st, 128)], wv_sb[:, cc, :],
                            start=(cc == 0), stop=False)
                    nc.tensor.matmul(vps[:], ones128_sb[:], bv1_sb[:],
                                     start=False, stop=True)
                    nc.vector.tensor_copy(Vp[:, tt, :], vps[:])

                for st in range(spk):
                    ops_l.append(lambda s=st: v_set(s))
                return ops_l

            # ---- output-projection ops for one t-chunk ----
            def proj_ops(tci):
                def do_tile(tt):
                    po1 = ops.tile([128, 512], f32, tag="oT", name="po1")
                    po2 = ops.tile([128, 256], f32, tag="oT", name="po2")
                    for po, cs, cw in ((po1, 0, 512), (po2, 512, 256)):
                        nc.tensor.matmul(po[:], yTa[:, ts(tt, 128)],
                                         wpA_sb[:, ds(cs, cw)],
                                         start=True, stop=False)
                        nc.tensor.matmul(po[:], yT2[:, ts(tt, 128)],
                                         wpB_sb[:, ds(cs, cw)],
                                         start=False, stop=True)
                    pout = xin.tile([128, C], f32, tag="pout", name="pout")
                    nc.vector.tensor_copy(pout[:, 0:512], po1[:])
                    nc.vector.tensor_copy(pout[:, 512:768], po2[:])
                    nc.sync.dma_start(out_d[ts(tt, 128), :], pout[:])

                return [lambda x=(tci * spk + s): do_tile(x)
                        for s in range(spk)]

            # ---- attention ----
            def normalize(oT, h, qc):
                den = wkp.tile([1, TCH], f32, tag="den", name="den")
                nc.vector.tensor_copy(den[:], oT[64:65, :])
                recip = wkp.tile([1, TCH], f32, tag="recip", name="recip")
                nc.vector.reciprocal_approx_fast(out=recip[:], in_=den[:])
                rb = wkp.tile([64, TCH], f32, tag="rb", name="rb")
                nc.gpsimd.partition_broadcast(rb[:], recip[:])
                if h == 0:
                    nc.vector.tensor_mul(yTa[0:64, ts(qc, TCH)], oT[0:64, :],
                                         rb[:])
                elif h == 2:
                    nc.vector.tensor_mul(yT2[0:64, ts(qc, TCH)], oT[0:64, :],
                                         rb[:])
                else:
                    y1t = wkp.tile([64, TCH], bf16, tag="y1t", name="y1t")
                    nc.vector.tensor_mul(y1t[:], oT[0:64, :], rb[:])
                    nc.sync.dma_start(yTa[64:128, ts(qc, TCH)], y1t[:])

            def attention(qc, fillers):
                nkc = (qc + 1) * spk
                q0 = qc * TCH

                def lo_of(kc):
                    m = kc - qc * spk
                    return max(0, 128 * m), m

                # ---- pass 1: heads 0,1 row-tiled ----
                oT0 = ops.tile([65, TCH], f32, tag="oT", name="oT0")
                oT1 = ops.tile([65, TCH], f32, tag="oT", name="oT1")
                s_pend = {}

                def emit_s01(kc):
                    lo, _ = lo_of(kc)
                    S = sps.tile([128, 1024], f32, tag="S", name="S01")
                    nc.tensor.matmul(
                        S[:, lo:TCH],
                        K01[0:64, ts(kc, KCH)], Q01[0:64, ds(q0 + lo,
                                                             TCH - lo)],
                        start=True, stop=True, tile_position=(0, 0))
                    nc.tensor.matmul(
                        S[:, TCH + lo:1024],
                        K01[64:128, ts(kc, KCH)], Q01[64:128, ds(q0 + lo,
                                                                 TCH - lo)],
                        start=True, stop=True, tile_position=(64, 0))
                    s_pend[kc] = S

                emit_s01(0)
                if nkc > 1:
                    emit_s01(1)
                for kc in range(nkc):
                    lo, m = lo_of(kc)
                    S = s_pend.pop(kc)
                    PT = ptp.tile([128, 1024], bf16, tag="PT", name="PT")
                    if lo == 0:
                        nc.scalar.activation(PT[:], S[:], AF.Exp, scale=0.125)
                    else:
                        nc.scalar.activation(PT[:, lo:TCH], S[:, lo:TCH],
                                             AF.Exp, scale=0.125)
                        nc.scalar.activation(PT[:, TCH + lo:1024],
                                             S[:, TCH + lo:1024],
                                             AF.Exp, scale=0.125)
                    if kc + 2 < nkc:
                        emit_s01(kc + 2)
                    if m >= 0:
                        nc.vector.tensor_mul(PT[:, ds(lo, 128)],
                                             PT[:, ds(lo, 128)], tri_sb[:])
                        nc.vector.tensor_mul(PT[:, ds(TCH + lo, 128)],
                                             PT[:, ds(TCH + lo, 128)],
                                             tri_sb[:])
                    nc.tensor.matmul(oT0[:, lo:TCH], Vp[:, kc, 0:65],
                                     PT[:, lo:TCH],
                                     start=(kc == 0), stop=(kc == nkc - 1))
                    nc.tensor.matmul(oT1[:, lo:TCH], Vp[:, kc, 65:130],
                                     PT[:, TCH + lo:1024],
                                     start=(kc == 0), stop=(kc == nkc - 1))
                    if fillers:
                        fillers.pop(0)()
                normalize(oT0, 0, qc)
                normalize(oT1, 1, qc)

                # ---- pass 2: head 2, k-chunk pairs row-tiled ----
                oT2 = ops.tile([65, TCH], f32, tag="oT", name="oT2")
                npair = nkc // 2
                s2_pend = {}

                def emit_s2(kp):
                    kc0, kc1 = 2 * kp, 2 * kp + 1
                    lo0, _ = lo_of(kc0)
                    lo1, _ = lo_of(kc1)
                    S2 = sps.tile([128, 1024], f32, tag="S", name="S2")
                    nc.tensor.matmul(
                        S2[:, lo0:TCH],
                        K2[0:64, ts(kc0, KCH)], Q2[0:64, ds(q0 + lo0,
                                                            TCH - lo0)],
                        start=True, stop=True, tile_position=(0, 0))
                    nc.tensor.matmul(
                        S2[:, TCH + lo1:1024],
                        K2[64:128, ts(kc1, KCH)], Q2[64:128, ds(q0 + lo1,
                                                                TCH - lo1)],
                        start=True, stop=True, tile_position=(64, 0))
                    s2_pend[kp] = S2

                emit_s2(0)
                if npair > 1:
                    emit_s2(1)
                for kp in range(npair):
                    kc0, kc1 = 2 * kp, 2 * kp + 1
                    lo0, m0 = lo_of(kc0)
                    lo1, m1 = lo_of(kc1)
                    S2 = s2_pend.pop(kp)
                    PT2 = ptp.tile([128, 1024], bf16, tag="PT", name="PT2")
                    if lo0 == 0 and lo1 == 0:
                        nc.scalar.activation(PT2[:], S2[:], AF.Exp,
                                             scale=0.125)
                    else:
                        nc.scalar.activation(PT2[:, lo0:TCH], S2[:, lo0:TCH],
                                             AF.Exp, scale=0.125)
                        nc.scalar.activation(PT2[:, TCH + lo1:1024],
                                             S2[:, TCH + lo1:1024],
                                             AF.Exp, scale=0.125)
                    if kp + 2 < npair:
                        emit_s2(kp + 2)
                    if m0 >= 0:
                        nc.vector.tensor_mul(PT2[:, ds(lo0, 128)],
                                             PT2[:, ds(lo0, 128)], tri_sb[:])
                    if m1 >= 0:
                        nc.vector.tensor_mul(PT2[:, ds(TCH + lo1, 128)],
                                             PT2[:, ds(TCH + lo1, 128)],
                                             tri_sb[:])
                    nc.tensor.matmul(oT2[:, lo0:TCH], Vp[:, kc0, 130:195],
                                     PT2[:, lo0:TCH],
                                     start=(kp == 0), stop=False)
                    nc.tensor.matmul(oT2[:, lo1:TCH], Vp[:, kc1, 130:195],
                                     PT2[:, TCH + lo1:1024],
                                     start=False, stop=(kp == npair - 1))
                    if fillers:
                        fillers.pop(0)()
                normalize(oT2, 2, qc)

            # ---- main schedule: QKV(0) up front, then per-qc attention
            # with next-chunk QKV + prev-chunk proj injected as PE fillers
            for op in qkv_ops(0):
                op()
            for qc in range(nt):
                fillers = []
                if qc + 1 < nt:
                    fillers += qkv_ops(qc + 1)
                if qc >= 1:
                    fillers += proj_ops(qc - 1)
                attention(qc, fillers)
                for op in fillers:
                    op()
            for op in proj_ops(nt - 1):
                op()

    return nc


def make_tri():
    import ml_dtypes
    p = np.arange(128)[:, None]
    j = np.arange(128)[None, :]
    return (j - p >= 0).astype(ml_dtypes.bfloat16)


def core_inputs(c, x, w_attn, b_attn, w_proj, xT_by_batch, tri):
    import ml_dtypes
    f32 = np.float32
    b = c // 4
    heads = [(c % 4) * HPC + i for i in range(HPC)]
    h0, h1, h2 = heads

    def Wq(h):
        return w_attn[:, h * HD:(h + 1) * HD]

    def Wk(h):
        return w_attn[:, C + h * HD:C + (h + 1) * HD]

    def Wv(h):
        return w_attn[:, 2 * C + h * HD:2 * C + (h + 1) * HD]

    def bq(h):
        return b_attn[h * HD:(h + 1) * HD]

    def bk(h):
        return b_attn[C + h * HD:C + (h + 1) * HD]

    def bv(h):
        return b_attn[2 * C + h * HD:2 * C + (h + 1) * HD]

    wv195 = np.zeros((C, 195), f32)
    bv1 = np.zeros((1, 195), f32)
    for i, h in enumerate(heads):
        wv195[:, i * 65:i * 65 + 64] = Wv(h)
        bv1[0, i * 65:i * 65 + 64] = bv(h)
        bv1[0, i * 65 + 64] = 1.0
    bf = ml_dtypes.bfloat16

    def arr(w):
        m = w.shape[1]
        return np.ascontiguousarray(
            w.reshape(C // CPART, CPART, m).transpose(1, 0, 2).reshape(
                CPART, -1)).astype(bf)

    return {
        "xT": xT_by_batch[b],
        "wq01": arr(np.concatenate([Wq(h0), Wq(h1)], 1)),
        "wk01": arr(np.concatenate([Wk(h0), Wk(h1)], 1)),
        "wqk2": arr(np.concatenate([Wq(h2), Wk(h2)], 1)),
        "wv": arr(wv195),
        "bv1": bv1.astype(ml_dtypes.bfloat16),
        "bq01": np.ascontiguousarray(
            np.concatenate([bq(h0), bq(h1)])[:, None]),
        "bk01": np.ascontiguousarray(
            np.concatenate([bk(h0), bk(h1)])[:, None]),
        "bqk2": np.ascontiguousarray(
            np.concatenate([bq(h2), bk(h2)])[:, None]),
        "wp": np.ascontiguousarray(
            np.concatenate([w_proj[h * HD:(h + 1) * HD, :] for h in heads],
                           0)).astype(bf),
        "tri": tri,
        "ones128": np.ones((1, 128), ml_dtypes.bfloat16),
    }


TRACE = False
LAST_EXEC_NS = None
LAST_RESULTS = None


def kernel(x, w_attn, b_attn, w_proj, b_proj):
    global LAST_EXEC_NS, LAST_RESULTS
    _ensure_axon_hooks_module()
    from concourse.bass_utils import run_bass_kernel_spmd

    x = np.asarray(x, np.float32)
    w_attn = np.asarray(w_attn, np.float32)
    b_attn = np.asarray(b_attn, np.float32)
    w_proj = np.asarray(w_proj, np.float32)
    b_proj = np.asarray(b_proj, np.float32)

    if "nc" not in _cache:
        nc = build_program()
        nc.finalize()
        _cache["nc"] = nc
    nc = _cache["nc"]

    import ml_dtypes
    xT_by_batch = [np.ascontiguousarray(x[b].T).astype(ml_dtypes.bfloat16)
                   for b in range(B)]
    tri = make_tri()
    in_maps = [
        core_inputs(c, x, w_attn, b_attn, w_proj, xT_by_batch, tri)
        for c in range(N_CORES)
    ]
    res = run_bass_kernel_spmd(nc, in_maps, core_ids=list(range(N_CORES)),
                               trace=TRACE)
    LAST_EXEC_NS = res.exec_time_ns
    LAST_RESULTS = res
    out = np.zeros((B, T, C), np.float32)
    for c in range(N_CORES):
        out[c // 4] += np.asarray(res.results[c]["out"], np.float32)
    out += b_proj[None, None, :]
    return out


# revision 14
# speedup vs baseline: 1.7726x; 1.0095x over previous
"""Causal self-attention (GPT-style, B=2, T=4096, C=768, 12 heads) on 8 TRN2
NeuronCores.

Sharding: core c handles batch b = c//4 and heads [3g, 3g+1, 3g+2] with
g = c%4 (data parallel on B x tensor parallel on heads).  Each core computes
its heads' attention output projected through its slice of w_proj; the host
sums the 4 partial [T, C] outputs per batch and adds b_proj.

Device-side formulation (all matmuls bf16, fp32 accumulate):
  - host passes x[b].T so QKV projections contract C on partitions:
      qT/kT  = W.T @ x.T        -> [head_dim(=partitions), T]
      V'     = x @ [Wv|0] + ones-col -> [T(=partitions), 3*65]  (col 64 of
               each 65-block is constant 1 -> PV also yields softmax denoms)
  - scores computed transposed, S^T[k, q] via lhsT=kT, rhs=qT; two heads per
    512-cycle slot via PE row-tiling (K=64 each, concurrent row groups).
  - softmax without max-subtraction (scores are O(5), exp safe in fp32):
      P^T = exp(0.125 * S^T) on ScalarE, PSUM->SBUF bf16, one activate per
      [128, 1024] (both heads / both k-chunks of a slot share it).
  - causal: strictly-masked k-chunks never computed; at boundary tiles the
    score matmul / exp / PV shrink to the valid column subrange and only the
    diagonal [128,128] strip is multiplied by a triangular 0/1 bf16 mask.
  - PV: oT'[65, q] += V'[k,65].T @ P^T[k,q] accumulated over k-chunks; row 64
    is the softmax denominator.  Normalize: approx-reciprocal on DVE,
    partition-broadcast via a stride-0 SBUF->SBUF DMA, multiply -> yT bf16.
  - output projection (packed K=128): out[t,:] = yTa[:,t].T @ wp[h01-rows] +
    yT2[:,t].T @ wp[h2-rows];  h1's normalized slab is DMA-shifted into
    partitions 64..127 of yTa so two heads contract in one matmul.
  - QKV-projection and output-projection work is interleaved into the
    attention loop as PE filler ops so TensorE never idles (keeps the HAM
    clock gate at 2.4 GHz) while ScalarE streams the exps.
"""

import numpy as np

N_CORES = 8
B = 2
T = 4096
C = 768
NH = 12
HD = 64
HPC = 3            # heads per core
TCH = 512          # t / q chunk
KCH = 128          # k chunk
CPART = 128

_cache = {}


def _ensure_axon_hooks_module():
    """Make `from antenv.axon_hooks import ...` importable even on images
    whose antenv package lacks the module (profiling then degrades to a
    no-op instead of crashing run_bass_kernel_spmd(trace=True))."""
    import sys
    import types
    try:
        import antenv.axon_hooks  # noqa: F401
        return
    except Exception:
        pass
    m = types.ModuleType("antenv.axon_hooks")
    m._hook = None

    def _set(h):
        m._hook = h

    def _get():
        return m._hook

    m.set_axon_ntff_profile_hook = _set
    m.get_axon_ntff_profile_hook = _get
    sys.modules["antenv.axon_hooks"] = m


def build_program(t=T):
    """Build the single-core SPMD bass program (same program on all cores,
    per-core data). Returns the un-finalized Bacc."""
    import concourse.mybir as mybir
    import concourse.tile as tile
    from concourse import bacc
    from concourse.bass import ds, ts

    f32 = mybir.dt.float32
    bf16 = mybir.dt.bfloat16
    AF = mybir.ActivationFunctionType

    nt = t // TCH          # number of t/q chunks
    spk = TCH // KCH       # k-chunks per t-chunk (4)
    cc_n = C // CPART      # 6 contraction chunks

    nc = bacc.Bacc("TRN2", target_bir_lowering=False)

    # packed bf16 constants: [wq01 768 | wk01 768 | wqk2 768 | wv 1170 |
    #  wpA 768 | wpB 768 (rows 0:64) | tri 128 | misc 384 (row0: bv1+ones128)]
    PK_W = 6 * 128 * 3 + 6 * 195 + C + C + 128 + 384
    xT = nc.dram_tensor("xT", [128, (t // TCH) * (C // CPART) * TCH], bf16,
                        kind="ExternalInput")
    wpk_d = nc.dram_tensor("wpk", [128, PK_W], bf16, kind="ExternalInput")
    bpk_d = nc.dram_tensor("bpk", [128, 3], f32, kind="ExternalInput")
    out_d = nc.dram_tensor("out", [t, C], f32, kind="ExternalOutput")

    with tile.TileContext(nc) as tc_:
        with (
            tc_.tile_pool(name="consts", bufs=1) as consts,
            tc_.tile_pool(name="big", bufs=1) as big,
            tc_.tile_pool(name="xin", bufs=2) as xin,
            tc_.tile_pool(name="ptp", bufs=6) as ptp,
            tc_.tile_pool(name="wkp", bufs=3) as wkp,
            tc_.tile_pool(name="sps", bufs=2, space="PSUM") as sps,
            tc_.tile_pool(name="ops", bufs=4, space="PSUM") as ops,
        ):
            # ---- init: one packed bf16 weight DMA + one f32 bias DMA ----
            wpk = consts.tile([128, PK_W], bf16)
            nc.sync.dma_start(wpk[:], wpk_d[:, :])
            bpk = consts.tile([128, 3], f32)
            nc.sync.dma_start(bpk[:], bpk_d[:, :])

            def seg(off, w):
                ap = wpk[:, off:off + w]
                return ap, off + w

            _o = 0
            wq01_f, _o = seg(_o, 6 * 128)
            wk01_f, _o = seg(_o, 6 * 128)
            wqk2_f, _o = seg(_o, 6 * 128)
            wv_f, _o = seg(_o, 6 * 195)
            wpA_sb, _o = seg(_o, C)
            wpB_full, _o = seg(_o, C)
            tri_sb, _o = seg(_o, 128)
            misc_f, _o = seg(_o, 384)
            wq01_sb = wq01_f.rearrange("p (c m) -> p c m", c=cc_n)
            wk01_sb = wk01_f.rearrange("p (c m) -> p c m", c=cc_n)
            wqk2_sb = wqk2_f.rearrange("p (c m) -> p c m", c=cc_n)
            wv_sb = wv_f.rearrange("p (c m) -> p c m", c=cc_n)
            wpB_sb = wpB_full[0:64, :]
            bv1_sb = misc_f[0:1, 0:195]
            ones128_sb = misc_f[0:1, 195:195 + 128]
            bq01_sb = bpk[:, 0:1]
            bk01_sb = bpk[:, 1:2]
            bqk2_sb = bpk[:, 2:3]

            # ---- persistent activations ----
            Q01 = big.tile([128, t], bf16)   # rows 0-63 qT_h0, 64-127 qT_h1
            K01 = big.tile([128, t], bf16)
            Q2 = big.tile([128, t], bf16)    # qT_h2 duplicated on both halves
            K2 = big.tile([128, t], bf16)
            Vp = big.tile([128, t // KCH, 195], bf16)
            yTa = big.tile([128, t], bf16)   # normalized h0 (0:64) | h1
            yT2 = big.tile([64, t], bf16)

            xT_r = xT[:, :].rearrange("p (nt c m) -> p nt c m", nt=nt,
                                      c=cc_n)

            # ---- QKV projection ops for one t-chunk (list of closures) ----
            def qkv_ops(tci):
                state = {}
                ops_l = []

                def dma_cast():
                    xtb = xin.tile([128, cc_n, TCH], bf16, tag="xtb",
                                   name="xtb")
                    nc.sync.dma_start(xtb[:], xT_r[:, tci, :, :])
                    state["xtb"] = xtb
                ops_l.append(dma_cast)

                def qk_set(wsb, bsb, dst):
                    xtb = state["xtb"]
                    qkps = sps.tile([128, TCH], f32, tag="S", name="qkps")
                    for cc in range(cc_n):
                        nc.tensor.matmul(
                            qkps[:], wsb[:, cc, :], xtb[:, cc, :],
                            start=(cc == 0), stop=(cc == cc_n - 1))
                    if dst is None:
                        # packed [qT_h2; kT_h2]: bias-add the aligned halves
                        # into Q2/K2, then DMA-duplicate across halves.
                        nc.vector.tensor_scalar_add(
                            Q2[0:64, ts(tci, TCH)], qkps[0:64, :],
                            bsb[0:64, :])
                        nc.vector.tensor_scalar_add(
                            K2[64:128, ts(tci, TCH)], qkps[64:128, :],
                            bsb[64:128, :])
                        nc.sync.dma_start(Q2[64:128, ts(tci, TCH)],
                                          Q2[0:64, ts(tci, TCH)])
                        nc.sync.dma_start(K2[0:64, ts(tci, TCH)],
                                          K2[64:128, ts(tci, TCH)])
                    else:
                        nc.vector.tensor_scalar_add(
                            dst[:, ts(tci, TCH)], qkps[:], bsb[:])

                for wsb, bsb, dst in (
                    (wq01_sb, bq01_sb, Q01),
                    (wk01_sb, bk01_sb, K01),
                    (wqk2_sb, bqk2_sb, None),
                ):
                    ops_l.append(
                        lambda w=wsb, b=bsb, d=dst: qk_set(w, b, d))

                def v_set(st):
                    xtb = state["xtb"]
                    tt = tci * spk + st
                    vps = ops.tile([128, 195], f32, tag="oT", name="vps")
                    for cc in range(cc_n):
                        nc.tensor.matmul(
                            vps[:], xtb[:, cc, ts(st, 128)], wv_sb[:, cc, :],
                            start=(cc == 0), stop=False)
                    nc.tensor.matmul(vps[:], ones128_sb[:], bv1_sb[:],
                                     start=False, stop=True)
                    nc.vector.tensor_copy(Vp[:, tt, :], vps[:])

                for st in range(spk):
                    ops_l.append(lambda s=st: v_set(s))
                return ops_l

            # ---- output-projection ops for one t-chunk ----
            def proj_ops(tci):
                def do_tile(tt):
                    po1 = ops.tile([128, 512], f32, tag="oT", name="po1")
                    po2 = ops.tile([128, 256], f32, tag="oT", name="po2")
                    for po, cs, cw in ((po1, 0, 512), (po2, 512, 256)):
                        nc.tensor.matmul(po[:], yTa[:, ts(tt, 128)],
                                         wpA_sb[:, ds(cs, cw)],
                                         start=True, stop=False)
                        nc.tensor.matmul(po[:], yT2[:, ts(tt, 128)],
                                         wpB_sb[:, ds(cs, cw)],
                                         start=False, stop=True)
                    pout = xin.tile([128, C], f32, tag="pout", name="pout")
                    nc.vector.tensor_copy(pout[:, 0:512], po1[:])
                    nc.vector.tensor_copy(pout[:, 512:768], po2[:])
                    nc.sync.dma_start(out_d[ts(tt, 128), :], pout[:])

                return [lambda x=(tci * spk + s): do_tile(x)
                        for s in range(spk)]

            # ---- attention ----
            def normalize(oT, h, qc):
                den = wkp.tile([1, TCH], f32, tag="den", name="den")
                nc.vector.tensor_copy(den[:], oT[64:65, :])
                recip = wkp.tile([1, TCH], f32, tag="recip", name="recip")
                nc.vector.reciprocal_approx_fast(out=recip[:], in_=den[:])
                rb = wkp.tile([64, TCH], f32, tag="rb", name="rb")
                nc.gpsimd.partition_broadcast(rb[:], recip[:])
                if h == 0:
                    nc.vector.tensor_mul(yTa[0:64, ts(qc, TCH)], oT[0:64, :],
                                         rb[:])
                elif h == 2:
                    nc.vector.tensor_mul(yT2[0:64, ts(qc, TCH)], oT[0:64, :],
                                         rb[:])
                else:
                    y1t = wkp.tile([64, TCH], bf16, tag="y1t", name="y1t")
                    nc.vector.tensor_mul(y1t[:], oT[0:64, :], rb[:])
                    nc.sync.dma_start(yTa[64:128, ts(qc, TCH)], y1t[:])

            def attention(qc, fillers):
                nkc = (qc + 1) * spk
                q0 = qc * TCH

                def lo_of(kc):
                    m = kc - qc * spk
                    return max(0, 128 * m), m

                # ---- pass 1: heads 0,1 row-tiled ----
                oT0 = ops.tile([65, TCH], f32, tag="oT", name="oT0")
                oT1 = ops.tile([65, TCH], f32, tag="oT", name="oT1")
                s_pend = {}

                def emit_s01(kc):
                    lo, _ = lo_of(kc)
                    S = sps.tile([128, 1024], f32, tag="S", name="S01")
                    nc.tensor.matmul(
                        S[:, lo:TCH],
                        K01[0:64, ts(kc, KCH)], Q01[0:64, ds(q0 + lo,
                                                             TCH - lo)],
                        start=True, stop=True, tile_position=(0, 0))
                    nc.tensor.matmul(
                        S[:, TCH + lo:1024],
                        K01[64:128, ts(kc, KCH)], Q01[64:128, ds(q0 + lo,
                                                                 TCH - lo)],
                        start=True, stop=True, tile_position=(64, 0))
                    s_pend[kc] = S

                emit_s01(0)
                if nkc > 1:
                    emit_s01(1)
                for kc in range(nkc):
                    lo, m = lo_of(kc)
                    S = s_pend.pop(kc)
                    PT = ptp.tile([128, 1024], bf16, tag="PT", name="PT")
                    if lo == 0:
                        nc.scalar.activation(PT[:], S[:], AF.Exp, scale=0.125)
                    else:
                        nc.scalar.activation(PT[:, lo:TCH], S[:, lo:TCH],
                                             AF.Exp, scale=0.125)
                        nc.scalar.activation(PT[:, TCH + lo:1024],
                                             S[:, TCH + lo:1024],
                                             AF.Exp, scale=0.125)
                    if kc + 2 < nkc:
                        emit_s01(kc + 2)
                    if m >= 0:
                        nc.vector.tensor_mul(PT[:, ds(lo, 128)],
                                             PT[:, ds(lo, 128)], tri_sb[:])
                        nc.vector.tensor_mul(PT[:, ds(TCH + lo, 128)],
                                             PT[:, ds(TCH + lo, 128)],
                                             tri_sb[:])
                    nc.tensor.matmul(oT0[:, lo:TCH], Vp[:, kc, 0:65],
                                     PT[:, lo:TCH],
                                     start=(kc == 0), stop=(kc == nkc - 1))
                    nc.tensor.matmul(oT1[:, lo:TCH], Vp[:, kc, 65:130],
                                     PT[:, TCH + lo:1024],
                                     start=(kc == 0), stop=(kc == nkc - 1))
                    if fillers:
                        fillers.pop(0)()
                normalize(oT0, 0, qc)
                normalize(oT1, 1, qc)

                # ---- pass 2: head 2, k-chunk pairs row-tiled ----
                oT2 = ops.tile([65, TCH], f32, tag="oT", name="oT2")
                npair = nkc // 2
                s2_pend = {}

                def emit_s2(kp):
                    kc0, kc1 = 2 * kp, 2 * kp + 1
                    lo0, _ = lo_of(kc0)
                    lo1, _ = lo_of(kc1)
                    S2 = sps.tile([128, 1024], f32, tag="S", name="S2")
                    nc.tensor.matmul(
                        S2[:, lo0:TCH],
                        K2[0:64, ts(kc0, KCH)], Q2[0:64, ds(q0 + lo0,
                                                            TCH - lo0)],
                        start=True, stop=True, tile_position=(0, 0))
                    nc.tensor.matmul(
                        S2[:, TCH + lo1:1024],
                        K2[64:128, ts(kc1, KCH)], Q2[64:128, ds(q0 + lo1,
                                                                TCH - lo1)],
                        start=True, stop=True, tile_position=(64, 0))
                    s2_pend[kp] = S2

                emit_s2(0)
                if npair > 1:
                    emit_s2(1)
                for kp in range(npair):
                    kc0, kc1 = 2 * kp, 2 * kp + 1
                    lo0, m0 = lo_of(kc0)
                    lo1, m1 = lo_of(kc1)
                    S2 = s2_pend.pop(kp)
                    PT2 = ptp.tile([128, 1024], bf16, tag="PT", name="PT2")
                    if lo0 == 0 and lo1 == 0:
                        nc.scalar.activation(PT2[:], S2[:], AF.Exp,
                                             scale=0.125)
                    else:
                        nc.scalar.activation(PT2[:, lo0:TCH], S2[:, lo0:TCH],
                                             AF.Exp, scale=0.125)
                        nc.scalar.activation(PT2[:, TCH + lo1:1024],
                                             S2[:, TCH + lo1:1024],
                                             AF.Exp, scale=0.125)
                    if kp + 2 < npair:
                        emit_s2(kp + 2)
                    if m0 >= 0:
                        nc.vector.tensor_mul(PT2[:, ds(lo0, 128)],
                                             PT2[:, ds(lo0, 128)], tri_sb[:])
                    if m1 >= 0:
                        nc.vector.tensor_mul(PT2[:, ds(TCH + lo1, 128)],
                                             PT2[:, ds(TCH + lo1, 128)],
                                             tri_sb[:])
                    nc.tensor.matmul(oT2[:, lo0:TCH], Vp[:, kc0, 130:195],
                                     PT2[:, lo0:TCH],
                                     start=(kp == 0), stop=False)
                    nc.tensor.matmul(oT2[:, lo1:TCH], Vp[:, kc1, 130:195],
                                     PT2[:, TCH + lo1:1024],
                                     start=False, stop=(kp == npair - 1))
                    if fillers:
                        fillers.pop(0)()
                normalize(oT2, 2, qc)

            # ---- main schedule: QKV(0) up front, then per-qc attention
            # with next-chunk QKV + prev-chunk proj injected as PE fillers
            for op in qkv_ops(0):
                op()
            for qc in range(nt):
                fillers = []
                if qc + 1 < nt:
                    fillers += qkv_ops(qc + 1)
                if qc >= 1:
                    fillers += proj_ops(qc - 1)
                attention(qc, fillers)
                for op in fillers:
                    op()
            for op in proj_ops(nt - 1):
                op()

    return nc


def arrange_x(xb):
    """x[b] is [t, C]; device wants xT as [128, nt, cc, TCH] contiguous."""
    import ml_dtypes
    t = xb.shape[0]
    xt = xb.T.reshape(C // CPART, CPART, t // TCH, TCH)
    xt = xt.transpose(1, 2, 0, 3).reshape(CPART, -1)
    return np.ascontiguousarray(xt).astype(ml_dtypes.bfloat16)


def make_tri():
    import ml_dtypes
    p = np.arange(128)[:, None]
    j = np.arange(128)[None, :]
    return (j - p >= 0).astype(ml_dtypes.bfloat16)


def core_inputs(c, x, w_attn, b_attn, w_proj, xT_by_batch, tri):
    import ml_dtypes
    f32 = np.float32
    b = c // 4
    heads = [(c % 4) * HPC + i for i in range(HPC)]
    h0, h1, h2 = heads

    def Wq(h):
        return w_attn[:, h * HD:(h + 1) * HD]

    def Wk(h):
        return w_attn[:, C + h * HD:C + (h + 1) * HD]

    def Wv(h):
        return w_attn[:, 2 * C + h * HD:2 * C + (h + 1) * HD]

    def bq(h):
        return b_attn[h * HD:(h + 1) * HD]

    def bk(h):
        return b_attn[C + h * HD:C + (h + 1) * HD]

    def bv(h):
        return b_attn[2 * C + h * HD:2 * C + (h + 1) * HD]

    wv195 = np.zeros((C, 195), f32)
    bv1 = np.zeros((1, 195), f32)
    for i, h in enumerate(heads):
        wv195[:, i * 65:i * 65 + 64] = Wv(h)
        bv1[0, i * 65:i * 65 + 64] = bv(h)
        bv1[0, i * 65 + 64] = 1.0
    bf = ml_dtypes.bfloat16

    def arr(w):
        m = w.shape[1]
        return np.ascontiguousarray(
            w.reshape(C // CPART, CPART, m).transpose(1, 0, 2).reshape(
                CPART, -1)).astype(bf)

    wp192 = np.concatenate([w_proj[h * HD:(h + 1) * HD, :] for h in heads], 0)
    wpB = np.zeros((CPART, C), np.float32)
    wpB[0:64, :] = wp192[128:192, :]
    misc = np.zeros((CPART, 384), np.float32)
    misc[0, 0:195] = bv1[0]
    misc[0, 195:195 + 128] = 1.0
    wpk = np.concatenate([
        arr(np.concatenate([Wq(h0), Wq(h1)], 1)).astype(np.float32),
        arr(np.concatenate([Wk(h0), Wk(h1)], 1)).astype(np.float32),
        arr(np.concatenate([Wq(h2), Wk(h2)], 1)).astype(np.float32),
        arr(wv195).astype(np.float32),
        wp192[0:128, :], wpB, tri.astype(np.float32), misc,
    ], axis=1).astype(bf)
    bpk = np.stack([
        np.concatenate([bq(h0), bq(h1)]),
        np.concatenate([bk(h0), bk(h1)]),
        np.concatenate([bq(h2), bk(h2)]),
    ], axis=1).astype(np.float32)
    return {
        "xT": xT_by_batch[b],
        "wpk": np.ascontiguousarray(wpk),
        "bpk": np.ascontiguousarray(bpk),
    }


TRACE = False
LAST_EXEC_NS = None
LAST_RESULTS = None


def kernel(x, w_attn, b_attn, w_proj, b_proj):
    global LAST_EXEC_NS, LAST_RESULTS
    _ensure_axon_hooks_module()
    from concourse.bass_utils import run_bass_kernel_spmd

    x = np.asarray(x, np.float32)
    w_attn = np.asarray(w_attn, np.float32)
    b_attn = np.asarray(b_attn, np.float32)
    w_proj = np.asarray(w_proj, np.float32)
    b_proj = np.asarray(b_proj, np.float32)

    if "nc" not in _cache:
        nc = build_program()
        nc.finalize()
        _cache["nc"] = nc
    nc = _cache["nc"]

    import ml_dtypes  # noqa: F401
    xT_by_batch = [arrange_x(x[b]) for b in range(B)]
    tri = make_tri()
    in_maps = [
        core_inputs(c, x, w_attn, b_attn, w_proj, xT_by_batch, tri)
        for c in range(N_CORES)
    ]
    res = run_bass_kernel_spmd(nc, in_maps, core_ids=list(range(N_CORES)),
                               trace=TRACE)
    LAST_EXEC_NS = res.exec_time_ns
    LAST_RESULTS = res
    out = np.zeros((B, T, C), np.float32)
    for c in range(N_CORES):
        out[c // 4] += np.asarray(res.results[c]["out"], np.float32)
    out += b_proj[None, None, :]
    return out


# revision 15
# speedup vs baseline: 1.7787x; 1.0035x over previous
"""Causal self-attention (GPT-style, B=2, T=4096, C=768, 12 heads) on 8 TRN2
NeuronCores.

Sharding: core c handles batch b = c//4 and heads [3g, 3g+1, 3g+2] with
g = c%4 (data parallel on B x tensor parallel on heads).  Each core computes
its heads' attention output projected through its slice of w_proj; the host
sums the 4 partial [T, C] outputs per batch and adds b_proj.

Device-side formulation (all matmuls bf16, fp32 accumulate):
  - host passes x[b].T so QKV projections contract C on partitions:
      qT/kT  = W.T @ x.T        -> [head_dim(=partitions), T]
      V'     = x @ [Wv|0] + ones-col -> [T(=partitions), 3*65]  (col 64 of
               each 65-block is constant 1 -> PV also yields softmax denoms)
  - scores computed transposed, S^T[k, q] via lhsT=kT, rhs=qT; two heads per
    512-cycle slot via PE row-tiling (K=64 each, concurrent row groups).
  - softmax without max-subtraction (scores are O(5), exp safe in fp32):
      P^T = exp(0.125 * S^T) on ScalarE, PSUM->SBUF bf16, one activate per
      [128, 1024] (both heads / both k-chunks of a slot share it).
  - causal: strictly-masked k-chunks never computed; at boundary tiles the
    score matmul / exp / PV shrink to the valid column subrange and only the
    diagonal [128,128] strip is multiplied by a triangular 0/1 bf16 mask.
  - PV: oT'[65, q] += V'[k,65].T @ P^T[k,q] accumulated over k-chunks; row 64
    is the softmax denominator.  Normalize: approx-reciprocal on DVE,
    partition-broadcast via a stride-0 SBUF->SBUF DMA, multiply -> yT bf16.
  - output projection (packed K=128): out[t,:] = yTa[:,t].T @ wp[h01-rows] +
    yT2[:,t].T @ wp[h2-rows];  h1's normalized slab is DMA-shifted into
    partitions 64..127 of yTa so two heads contract in one matmul.
  - QKV-projection and output-projection work is interleaved into the
    attention loop as PE filler ops so TensorE never idles (keeps the HAM
    clock gate at 2.4 GHz) while ScalarE streams the exps.
"""

import numpy as np

N_CORES = 8
B = 2
T = 4096
C = 768
NH = 12
HD = 64
HPC = 3            # heads per core
TCH = 512          # t / q chunk
KCH = 128          # k chunk
CPART = 128

_cache = {}


def _ensure_axon_hooks_module():
    """Make `from antenv.axon_hooks import ...` importable even on images
    whose antenv package lacks the module (profiling then degrades to a
    no-op instead of crashing run_bass_kernel_spmd(trace=True))."""
    import sys
    import types
    try:
        import antenv.axon_hooks  # noqa: F401
        return
    except Exception:
        pass
    m = types.ModuleType("antenv.axon_hooks")
    m._hook = None

    def _set(h):
        m._hook = h

    def _get():
        return m._hook

    m.set_axon_ntff_profile_hook = _set
    m.get_axon_ntff_profile_hook = _get
    sys.modules["antenv.axon_hooks"] = m


def build_program(t=T):
    """Build the single-core SPMD bass program (same program on all cores,
    per-core data). Returns the un-finalized Bacc."""
    import concourse.mybir as mybir
    import concourse.tile as tile
    from concourse import bacc
    from concourse.bass import ds, ts

    f32 = mybir.dt.float32
    bf16 = mybir.dt.bfloat16
    AF = mybir.ActivationFunctionType

    nt = t // TCH          # number of t/q chunks
    spk = TCH // KCH       # k-chunks per t-chunk (4)
    cc_n = C // CPART      # 6 contraction chunks

    nc = bacc.Bacc("TRN2", target_bir_lowering=False)

    # packed bf16 constants: [wq01 768 | wk01 768 | wqk2 768 | wv 1170 |
    #  wpA 768 | wpB 768 (rows 0:64) | tri 128 | misc 384 (row0: bv1+ones128)]
    PK_W = 6 * 128 * 3 + 6 * 195 + C + C + 128 + 384
    xT = nc.dram_tensor("xT", [128, (t // TCH) * (C // CPART) * TCH], bf16,
                        kind="ExternalInput")
    wpk_d = nc.dram_tensor("wpk", [128, PK_W], bf16, kind="ExternalInput")
    bpk_d = nc.dram_tensor("bpk", [128, 3], f32, kind="ExternalInput")
    out_d = nc.dram_tensor("out", [t, C], f32, kind="ExternalOutput")

    with tile.TileContext(nc) as tc_:
        with (
            tc_.tile_pool(name="consts", bufs=1) as consts,
            tc_.tile_pool(name="big", bufs=1) as big,
            tc_.tile_pool(name="xin", bufs=2) as xin,
            tc_.tile_pool(name="ptp", bufs=6) as ptp,
            tc_.tile_pool(name="wkp", bufs=3) as wkp,
            tc_.tile_pool(name="sps", bufs=2, space="PSUM") as sps,
            tc_.tile_pool(name="ops", bufs=4, space="PSUM") as ops,
        ):
            # ---- init: packed weight DMAs (split so QKV weights land
            # first) + one f32 bias DMA ----
            wpk = consts.tile([128, PK_W], bf16)
            nc.sync.dma_start(wpk[:, 0:768], wpk_d[:, 0:768])
            nc.sync.dma_start(wpk[:, 768:2304], wpk_d[:, 768:2304])
            nc.sync.dma_start(wpk[:, 2304:3474], wpk_d[:, 2304:3474])
            nc.sync.dma_start(wpk[:, 3474:PK_W], wpk_d[:, 3474:PK_W])
            bpk = consts.tile([128, 3], f32)
            nc.sync.dma_start(bpk[:], bpk_d[:, :])

            def seg(off, w):
                ap = wpk[:, off:off + w]
                return ap, off + w

            _o = 0
            wq01_f, _o = seg(_o, 6 * 128)
            wk01_f, _o = seg(_o, 6 * 128)
            wqk2_f, _o = seg(_o, 6 * 128)
            wv_f, _o = seg(_o, 6 * 195)
            wpA_sb, _o = seg(_o, C)
            wpB_full, _o = seg(_o, C)
            tri_sb, _o = seg(_o, 128)
            misc_f, _o = seg(_o, 384)
            wq01_sb = wq01_f.rearrange("p (c m) -> p c m", c=cc_n)
            wk01_sb = wk01_f.rearrange("p (c m) -> p c m", c=cc_n)
            wqk2_sb = wqk2_f.rearrange("p (c m) -> p c m", c=cc_n)
            wv_sb = wv_f.rearrange("p (c m) -> p c m", c=cc_n)
            wpB_sb = wpB_full[0:64, :]
            bv1_sb = misc_f[0:1, 0:195]
            ones128_sb = misc_f[0:1, 195:195 + 128]
            bq01_sb = bpk[:, 0:1]
            bk01_sb = bpk[:, 1:2]
            bqk2_sb = bpk[:, 2:3]

            # ---- persistent activations ----
            Q01 = big.tile([128, t], bf16)   # rows 0-63 qT_h0, 64-127 qT_h1
            K01 = big.tile([128, t], bf16)
            Q2 = big.tile([128, t], bf16)    # qT_h2 duplicated on both halves
            K2 = big.tile([128, t], bf16)
            Vp = big.tile([128, t // KCH, 195], bf16)
            yTa = big.tile([128, t], bf16)   # normalized h0 (0:64) | h1
            yT2 = big.tile([64, t], bf16)

            xT_r = xT[:, :].rearrange("p (nt c m) -> p nt c m", nt=nt,
                                      c=cc_n)

            # ---- QKV projection ops for one t-chunk (list of closures) ----
            def qkv_ops(tci):
                state = {}
                ops_l = []

                def dma_cast():
                    xtb = xin.tile([128, cc_n, TCH], bf16, tag="xtb",
                                   name="xtb")
                    if tci == 0:
                        for cc in range(cc_n):
                            nc.sync.dma_start(xtb[:, cc, :],
                                              xT_r[:, tci, cc, :])
                    else:
                        nc.sync.dma_start(xtb[:], xT_r[:, tci, :, :])
                    state["xtb"] = xtb
                ops_l.append(dma_cast)

                def qk_set(wsb, bsb, dst):
                    xtb = state["xtb"]
                    qkps = sps.tile([128, TCH], f32, tag="S", name="qkps")
                    for cc in range(cc_n):
                        nc.tensor.matmul(
                            qkps[:], wsb[:, cc, :], xtb[:, cc, :],
                            start=(cc == 0), stop=(cc == cc_n - 1))
                    if dst is None:
                        # packed [qT_h2; kT_h2]: bias-add the aligned halves
                        # into Q2/K2, then DMA-duplicate across halves.
                        nc.vector.tensor_scalar_add(
                            Q2[0:64, ts(tci, TCH)], qkps[0:64, :],
                            bsb[0:64, :])
                        nc.vector.tensor_scalar_add(
                            K2[64:128, ts(tci, TCH)], qkps[64:128, :],
                            bsb[64:128, :])
                        nc.sync.dma_start(Q2[64:128, ts(tci, TCH)],
                                          Q2[0:64, ts(tci, TCH)])
                        nc.sync.dma_start(K2[0:64, ts(tci, TCH)],
                                          K2[64:128, ts(tci, TCH)])
                    else:
                        nc.vector.tensor_scalar_add(
                            dst[:, ts(tci, TCH)], qkps[:], bsb[:])

                for wsb, bsb, dst in (
                    (wq01_sb, bq01_sb, Q01),
                    (wk01_sb, bk01_sb, K01),
                    (wqk2_sb, bqk2_sb, None),
                ):
                    ops_l.append(
                        lambda w=wsb, b=bsb, d=dst: qk_set(w, b, d))

                def v_set(st):
                    xtb = state["xtb"]
                    tt = tci * spk + st
                    vps = ops.tile([128, 195], f32, tag="oT", name="vps")
                    for cc in range(cc_n):
                        nc.tensor.matmul(
                            vps[:], xtb[:, cc, ts(st, 128)], wv_sb[:, cc, :],
                            start=(cc == 0), stop=False)
                    nc.tensor.matmul(vps[:], ones128_sb[:], bv1_sb[:],
                                     start=False, stop=True)
                    nc.vector.tensor_copy(Vp[:, tt, :], vps[:])

                for st in range(spk):
                    ops_l.append(lambda s=st: v_set(s))
                return ops_l

            # ---- output-projection ops for one t-chunk ----
            def proj_ops(tci):
                def do_tile(tt):
                    po1 = ops.tile([128, 512], f32, tag="oT", name="po1")
                    po2 = ops.tile([128, 256], f32, tag="oT", name="po2")
                    for po, cs, cw in ((po1, 0, 512), (po2, 512, 256)):
                        nc.tensor.matmul(po[:], yTa[:, ts(tt, 128)],
                                         wpA_sb[:, ds(cs, cw)],
                                         start=True, stop=False)
                        nc.tensor.matmul(po[:], yT2[:, ts(tt, 128)],
                                         wpB_sb[:, ds(cs, cw)],
                                         start=False, stop=True)
                    pout = xin.tile([128, C], f32, tag="pout", name="pout")
                    nc.vector.tensor_copy(pout[:, 0:512], po1[:])
                    nc.vector.tensor_copy(pout[:, 512:768], po2[:])
                    nc.sync.dma_start(out_d[ts(tt, 128), :], pout[:])

                return [lambda x=(tci * spk + s): do_tile(x)
                        for s in range(spk)]

            # ---- attention ----
            def normalize(oT, h, qc):
                den = wkp.tile([1, TCH], f32, tag="den", name="den")
                nc.vector.tensor_copy(den[:], oT[64:65, :])
                recip = wkp.tile([1, TCH], f32, tag="recip", name="recip")
                nc.vector.reciprocal_approx_fast(out=recip[:], in_=den[:])
                rb = wkp.tile([64, TCH], f32, tag="rb", name="rb")
                nc.gpsimd.partition_broadcast(rb[:], recip[:])
                if h == 0:
                    nc.vector.tensor_mul(yTa[0:64, ts(qc, TCH)], oT[0:64, :],
                                         rb[:])
                elif h == 2:
                    nc.vector.tensor_mul(yT2[0:64, ts(qc, TCH)], oT[0:64, :],
                                         rb[:])
                else:
                    y1t = wkp.tile([64, TCH], bf16, tag="y1t", name="y1t")
                    nc.vector.tensor_mul(y1t[:], oT[0:64, :], rb[:])
                    nc.sync.dma_start(yTa[64:128, ts(qc, TCH)], y1t[:])

            def attention(qc, fillers):
                nkc = (qc + 1) * spk
                q0 = qc * TCH

                def lo_of(kc):
                    m = kc - qc * spk
                    return max(0, 128 * m), m

                # ---- pass 1: heads 0,1 row-tiled ----
                oT0 = ops.tile([65, TCH], f32, tag="oT", name="oT0")
                oT1 = ops.tile([65, TCH], f32, tag="oT", name="oT1")
                s_pend = {}

                def emit_s01(kc):
                    lo, _ = lo_of(kc)
                    S = sps.tile([128, 1024], f32, tag="S", name="S01")
                    nc.tensor.matmul(
                        S[:, lo:TCH],
                        K01[0:64, ts(kc, KCH)], Q01[0:64, ds(q0 + lo,
                                                             TCH - lo)],
                        start=True, stop=True, tile_position=(0, 0))
                    nc.tensor.matmul(
                        S[:, TCH + lo:1024],
                        K01[64:128, ts(kc, KCH)], Q01[64:128, ds(q0 + lo,
                                                                 TCH - lo)],
                        start=True, stop=True, tile_position=(64, 0))
                    s_pend[kc] = S

                emit_s01(0)
                if nkc > 1:
                    emit_s01(1)
                for kc in range(nkc):
                    lo, m = lo_of(kc)
                    S = s_pend.pop(kc)
                    PT = ptp.tile([128, 1024], bf16, tag="PT", name="PT")
                    if lo == 0:
                        nc.scalar.activation(PT[:], S[:], AF.Exp, scale=0.125)
                    else:
                        nc.scalar.activation(PT[:, lo:TCH], S[:, lo:TCH],
                                             AF.Exp, scale=0.125)
                        nc.scalar.activation(PT[:, TCH + lo:1024],
                                             S[:, TCH + lo:1024],
                                             AF.Exp, scale=0.125)
                    if kc + 2 < nkc:
                        emit_s01(kc + 2)
                    if m >= 0:
                        nc.vector.tensor_mul(PT[:, ds(lo, 128)],
                                             PT[:, ds(lo, 128)], tri_sb[:])
                        nc.vector.tensor_mul(PT[:, ds(TCH + lo, 128)],
                                             PT[:, ds(TCH + lo, 128)],
                                             tri_sb[:])
                    nc.tensor.matmul(oT0[:, lo:TCH], Vp[:, kc, 0:65],
                                     PT[:, lo:TCH],
                                     start=(kc == 0), stop=(kc == nkc - 1))
                    nc.tensor.matmul(oT1[:, lo:TCH], Vp[:, kc, 65:130],
                                     PT[:, TCH + lo:1024],
                                     start=(kc == 0), stop=(kc == nkc - 1))
                    if fillers:
                        fillers.pop(0)()
                normalize(oT0, 0, qc)
                normalize(oT1, 1, qc)

                # ---- pass 2: head 2, k-chunk pairs row-tiled ----
                oT2 = ops.tile([65, TCH], f32, tag="oT", name="oT2")
                npair = nkc // 2
                s2_pend = {}

                def emit_s2(kp):
                    kc0, kc1 = 2 * kp, 2 * kp + 1
                    lo0, _ = lo_of(kc0)
                    lo1, _ = lo_of(kc1)
                    S2 = sps.tile([128, 1024], f32, tag="S", name="S2")
                    nc.tensor.matmul(
                        S2[:, lo0:TCH],
                        K2[0:64, ts(kc0, KCH)], Q2[0:64, ds(q0 + lo0,
                                                            TCH - lo0)],
                        start=True, stop=True, tile_position=(0, 0))
                    nc.tensor.matmul(
                        S2[:, TCH + lo1:1024],
                        K2[64:128, ts(kc1, KCH)], Q2[64:128, ds(q0 + lo1,
                                                                TCH - lo1)],
                        start=True, stop=True, tile_position=(64, 0))
                    s2_pend[kp] = S2

                emit_s2(0)
                if npair > 1:
                    emit_s2(1)
                for kp in range(npair):
                    kc0, kc1 = 2 * kp, 2 * kp + 1
                    lo0, m0 = lo_of(kc0)
                    lo1, m1 = lo_of(kc1)
                    S2 = s2_pend.pop(kp)
                    PT2 = ptp.tile([128, 1024], bf16, tag="PT", name="PT2")
                    if lo0 == 0 and lo1 == 0:
                        nc.scalar.activation(PT2[:], S2[:], AF.Exp,
                                             scale=0.125)
                    else:
                        nc.scalar.activation(PT2[:, lo0:TCH], S2[:, lo0:TCH],
                                             AF.Exp, scale=0.125)
                        nc.scalar.activation(PT2[:, TCH + lo1:1024],
                                             S2[:, TCH + lo1:1024],
                                             AF.Exp, scale=0.125)
                    if kp + 2 < npair:
                        emit_s2(kp + 2)
                    if m0 >= 0:
                        nc.vector.tensor_mul(PT2[:, ds(lo0, 128)],
                                             PT2[:, ds(lo0, 128)], tri_sb[:])
                    if m1 >= 0:
                        nc.vector.tensor_mul(PT2[:, ds(TCH + lo1, 128)],
                                             PT2[:, ds(TCH + lo1, 128)],
                                             tri_sb[:])
                    nc.tensor.matmul(oT2[:, lo0:TCH], Vp[:, kc0, 130:195],
                                     PT2[:, lo0:TCH],
                                     start=(kp == 0), stop=False)
                    nc.tensor.matmul(oT2[:, lo1:TCH], Vp[:, kc1, 130:195],
                                     PT2[:, TCH + lo1:1024],
                                     start=False, stop=(kp == npair - 1))
                    if fillers:
                        fillers.pop(0)()
                normalize(oT2, 2, qc)

            # ---- main schedule: QKV(0) up front, then per-qc attention
            # with next-chunk QKV + prev-chunk proj injected as PE fillers
            for op in qkv_ops(0):
                op()
            for qc in range(nt):
                fillers = []
                if qc + 1 < nt:
                    fillers += qkv_ops(qc + 1)
                if qc >= 1:
                    fillers += proj_ops(qc - 1)
                attention(qc, fillers)
                for op in fillers:
                    op()
            for op in proj_ops(nt - 1):
                op()

    return nc


def arrange_x(xb):
    """x[b] is [t, C]; device wants xT as [128, nt, cc, TCH] contiguous."""
    import ml_dtypes
    t = xb.shape[0]
    xt = xb.T.reshape(C // CPART, CPART, t // TCH, TCH)
    xt = xt.transpose(1, 2, 0, 3).reshape(CPART, -1)
    return np.ascontiguousarray(xt).astype(ml_dtypes.bfloat16)


def make_tri():
    import ml_dtypes
    p = np.arange(128)[:, None]
    j = np.arange(128)[None, :]
    return (j - p >= 0).astype(ml_dtypes.bfloat16)


def core_inputs(c, x, w_attn, b_attn, w_proj, xT_by_batch, tri):
    import ml_dtypes
    f32 = np.float32
    b = c // 4
    heads = [(c % 4) * HPC + i for i in range(HPC)]
    h0, h1, h2 = heads

    def Wq(h):
        return w_attn[:, h * HD:(h + 1) * HD]

    def Wk(h):
        return w_attn[:, C + h * HD:C + (h + 1) * HD]

    def Wv(h):
        return w_attn[:, 2 * C + h * HD:2 * C + (h + 1) * HD]

    def bq(h):
        return b_attn[h * HD:(h + 1) * HD]

    def bk(h):
        return b_attn[C + h * HD:C + (h + 1) * HD]

    def bv(h):
        return b_attn[2 * C + h * HD:2 * C + (h + 1) * HD]

    wv195 = np.zeros((C, 195), f32)
    bv1 = np.zeros((1, 195), f32)
    for i, h in enumerate(heads):
        wv195[:, i * 65:i * 65 + 64] = Wv(h)
        bv1[0, i * 65:i * 65 + 64] = bv(h)
        bv1[0, i * 65 + 64] = 1.0
    bf = ml_dtypes.bfloat16

    def arr(w):
        m = w.shape[1]
        return np.ascontiguousarray(
            w.reshape(C // CPART, CPART, m).transpose(1, 0, 2).reshape(
                CPART, -1)).astype(bf)

    wp192 = np.concatenate([w_proj[h * HD:(h + 1) * HD, :] for h in heads], 0)
    wpB = np.zeros((CPART, C), np.float32)
    wpB[0:64, :] = wp192[128:192, :]
    misc = np.zeros((CPART, 384), np.float32)
    misc[0, 0:195] = bv1[0]
    misc[0, 195:195 + 128] = 1.0
    wpk = np.concatenate([
        arr(np.concatenate([Wq(h0), Wq(h1)], 1)).astype(np.float32),
        arr(np.concatenate([Wk(h0), Wk(h1)], 1)).astype(np.float32),
        arr(np.concatenate([Wq(h2), Wk(h2)], 1)).astype(np.float32),
        arr(wv195).astype(np.float32),
        wp192[0:128, :], wpB, tri.astype(np.float32), misc,
    ], axis=1).astype(bf)
    bpk = np.stack([
        np.concatenate([bq(h0), bq(h1)]),
        np.concatenate([bk(h0), bk(h1)]),
        np.concatenate([bq(h2), bk(h2)]),
    ], axis=1).astype(np.float32)
    return {
        "xT": xT_by_batch[b],
        "wpk": np.ascontiguousarray(wpk),
        "bpk": np.ascontiguousarray(bpk),
    }


TRACE = False
LAST_EXEC_NS = None
LAST_RESULTS = None


def kernel(x, w_attn, b_attn, w_proj, b_proj):
    global LAST_EXEC_NS, LAST_RESULTS
    _ensure_axon_hooks_module()
    from concourse.bass_utils import run_bass_kernel_spmd

    x = np.asarray(x, np.float32)
    w_attn = np.asarray(w_attn, np.float32)
    b_attn = np.asarray(b_attn, np.float32)
    w_proj = np.asarray(w_proj, np.float32)
    b_proj = np.asarray(b_proj, np.float32)

    if "nc" not in _cache:
        nc = build_program()
        nc.finalize()
        _cache["nc"] = nc
    nc = _cache["nc"]

    import ml_dtypes  # noqa: F401
    xT_by_batch = [arrange_x(x[b]) for b in range(B)]
    tri = make_tri()
    in_maps = [
        core_inputs(c, x, w_attn, b_attn, w_proj, xT_by_batch, tri)
        for c in range(N_CORES)
    ]
    res = run_bass_kernel_spmd(nc, in_maps, core_ids=list(range(N_CORES)),
                               trace=TRACE)
    LAST_EXEC_NS = res.exec_time_ns
    LAST_RESULTS = res
    out = np.zeros((B, T, C), np.float32)
    for c in range(N_CORES):
        out[c // 4] += np.asarray(res.results[c]["out"], np.float32)
    out += b_proj[None, None, :]
    return out
